# revision 25
# baseline (speedup 1.0000x reference)
"""GCN-GRU cell fused Trainium2 kernel (8-core data parallel), v2.

Math (per batch b):
    A = d * (adj+I).T * d,  d = rowsum(adj+I)^-0.5
    sig   = sigmoid(A @ [input, hidden] @ W1 + b1)   (N, 128)
    r, u  = node-split halves of sig (nodes <1024 / >=1024)
    c     = tanh(A @ [input, r*hidden] @ W2 + b2)
    out   = u * hidden + (1-u) * c

Structure (per core, 8 batches):
  - BG (big GEMM) feature-major: lhsT = X columns (features), rhs = A
    columns (nodes); PSUM out [feat, m], accumulated over 16 k-tiles.
  - W-GEMMs gate-major weight-stationary: lhsT = W (dup-row layout),
    rhs = AX chunk [feat, 512]; out [gates, m] in PSUM -> activation.
  - sig r-half bounced through DRAM + xbar dma transpose -> node-major
    for the x2 = r*x1 assembly (DVE).  Write+transpose share a queue
    (sync for even b, scalar for odd b) so FIFO order guarantees RAW.
  - W2 gates duplicated (cols doubled) so final gating runs f-major on
    DVE with stride-2 column views; out written f-major [f,b,t]; host
    does the final transpose to [b,n,h] (free).
  - Optional fp8e4 DoubleRow big GEMMs (FP8_BG1/FP8_BG2 env flags).
"""

import os
import numpy as np
import ml_dtypes
from contextlib import ExitStack

import concourse.bacc as bacc
import concourse.mybir as mybir
import concourse.tile as tile
from concourse.bass import ts, ds, broadcast_tensor_aps
from concourse.bass_utils import run_bass_kernel_spmd

P = 128
N = 2048
B = 64
H = 64
NCORES = 8
BL = B // NCORES          # 8 batches per core
KT = N // P               # 16 contraction tiles
CH = N // 512             # 4 output chunks of 512
F32 = mybir.dt.float32
BF16 = mybir.dt.bfloat16
FP8 = mybir.dt.float8e4
SIG = mybir.ActivationFunctionType.Sigmoid
TANH = mybir.ActivationFunctionType.Tanh
DR = mybir.MatmulPerfMode.DoubleRow

S_A = 16.0   # fp8 scale on A side
S_X = 32.0   # fp8 scale on X side

FP8_BG1 = os.environ.get("FP8_BG1", "1") == "1"
FP8_BG2 = os.environ.get("FP8_BG2", "1") == "1"

_CACHE = {}


def _build(fp8_bg1: bool, fp8_bg2: bool):
    nc = bacc.Bacc("TRN2", target_bir_lowering=False)

    any_bf = (not fp8_bg1) or (not fp8_bg2)
    any_f8 = fp8_bg1 or fp8_bg2
    two_dreps = fp8_bg1 != fp8_bg2

    # ---- dram tensors ----
    if any_bf:
        a_d = nc.dram_tensor("a", [N, N], BF16, kind="ExternalInput")
    if any_f8:
        a8_d = nc.dram_tensor("a8", [N, N], FP8, kind="ExternalInput")
    x1_d = nc.dram_tensor("x1", [N, BL * H], BF16, kind="ExternalInput")
    if fp8_bg1:
        x18_d = nc.dram_tensor("x18", [N, BL * H], FP8, kind="ExternalInput")
        xin8_d = nc.dram_tensor("xin8", [P, KT * 16], FP8, kind="ExternalInput")
    else:
        xin_d = nc.dram_tensor("xin", [P, KT * BL], BF16, kind="ExternalInput")
    drep1_d = nc.dram_tensor("drep1", [P, N], F32, kind="ExternalInput")
    if two_dreps:
        drep2_d = nc.dram_tensor("drep2", [P, N], F32, kind="ExternalInput")
    ht_d = nc.dram_tensor("ht", [P, BL, N // 2], BF16, kind="ExternalInput")
    w1h_d = nc.dram_tensor("w1h", [2 * H, 2 * H], BF16, kind="ExternalInput")
    w1i_d = nc.dram_tensor("w1i", [BL + 1, BL, 2 * H], BF16, kind="ExternalInput")
    w2h_d = nc.dram_tensor("w2h", [2 * H, 2 * H], BF16, kind="ExternalInput")
    w2i_d = nc.dram_tensor("w2i", [BL + 1, BL, 2 * H], BF16, kind="ExternalInput")
    out_d = nc.dram_tensor("out", [P, BL, N // 2], BF16, kind="ExternalOutput")
    # scratch for the r-half transpose bounce: one tensor per b so
    # whole-tensor DRAM dep tracking doesn't serialize across batches
    sigr_ds = [
        nc.dram_tensor(f"sigr{b}", [P, N // 2], BF16, kind="Internal")
        for b in range(BL)
    ]

    x2dt = FP8 if fp8_bg2 else BF16

    with tile.TileContext(nc) as tc, ExitStack() as ctx:
        const = ctx.enter_context(tc.tile_pool(name="const", bufs=1))
        x1_sb = const.tile([P, KT, BL * H], BF16)
        if fp8_bg1:
            x18_sb = const.tile([P, KT, BL * H], FP8)
            xin_sb = const.tile([P, KT, 16], FP8)
        else:
            xin_sb = const.tile([P, KT, BL], BF16)
        drep1_sb = const.tile([P, N], F32)
        drep2_sb = const.tile([P, N], F32) if two_dreps else drep1_sb
        ht_sb = const.tile([P, BL, N // 2], BF16)
        w1h_sb = const.tile([2 * H, 2 * H], BF16)
        w1i_sb = const.tile([BL + 1, BL, 2 * H], BF16)
        w2h_sb = const.tile([2 * H, 2 * H], BF16)
        w2i_sb = const.tile([BL + 1, BL, 2 * H], BF16)
        axin_sb = const.tile([BL + 1, N], BF16)     # rows 0..7 = d*(A@din), row 8 = ones
        axf1_sb = const.tile([P, CH, 4, 512], BF16)   # [2b x 64feat, ch, pair, m]
        axf2_sb = const.tile([P, CH, 4, 512], x2dt)
        sigu_sb = const.tile([P, BL, N // 2], BF16)    # [gate, b, m-1024]
        sigrt_sb = const.tile([P, KT, BL, H], BF16)    # node-major r gates
        x2_sb = const.tile([P, KT, BL * H], x2dt)
        c3_sb = const.tile([P, BL, N // 2], BF16)      # de-interleaved c, [f, b, t]

        x1_r = x1_d.ap().rearrange("(kt p) f -> p kt f", p=P)
        if fp8_bg1:
            x18_r = x18_d.ap().rearrange("(kt p) f -> p kt f", p=P)
            for g in range(8):
                nc.scalar.dma_start(x18_sb[:, ts(g, 2), :], x18_r[:, ts(g, 2), :])
            nc.scalar.dma_start(
                xin_sb[:], xin8_d.ap().rearrange("p (kt b) -> p kt b", b=16)
            )
        else:
            nc.scalar.dma_start(
                xin_sb[:], xin_d.ap().rearrange("p (kt b) -> p kt b", b=BL)
            )
        nc.scalar.dma_start(drep1_sb[:], drep1_d.ap())
        if two_dreps:
            nc.scalar.dma_start(drep2_sb[:], drep2_d.ap())
        nc.scalar.dma_start(w1h_sb[:], w1h_d.ap())
        nc.scalar.dma_start(w1i_sb[:], w1i_d.ap())
        nc.vector.memset(axin_sb[:], 1.0)
        for g in range(4):
            nc.scalar.dma_start(x1_sb[:, ts(g, 4), :], x1_r[:, ts(g, 4), :])

        both_f8 = fp8_bg1 and fp8_bg2
        if both_f8:
            a8_sb = const.tile([P, KT, N], FP8)
        apool = ctx.enter_context(tc.tile_pool(name="ap", bufs=3))
        spool = ctx.enter_context(tc.tile_pool(name="sp", bufs=4))
        gpool = ctx.enter_context(tc.tile_pool(name="gp", bufs=2))
        pps = ctx.enter_context(tc.tile_pool(name="ps", bufs=8, space="PSUM"))

        if any_bf:
            a_r = a_d.ap().rearrange("(kt p) m -> p kt m", p=P)
        if any_f8:
            a8_r = a8_d.ap().rearrange("(kt p) m -> p kt m", p=P)

        def load_a_chunk(ch, fp8, fine=False):
            if both_f8:
                at = a8_sb[:, :, ds(ch * 512, 512)]
                src = a8_r[:, :, ds(ch * 512, 512)]
                if fine:
                    for g in range(4):
                        nc.sync.dma_start(at[:, ts(g, 4), :], src[:, ts(g, 4), :])
                else:
                    nc.sync.dma_start(at[:], src)
                return at
            dt = FP8 if fp8 else BF16
            at = apool.tile([P, KT, 512], dt, tag="a8" if fp8 else "abf")
            src = (a8_r if fp8 else a_r)[:, :, ds(ch * 512, 512)]
            if fine:
                for g in range(4):
                    nc.sync.dma_start(at[:, ts(g, 4), :], src[:, ts(g, 4), :])
            else:
                nc.sync.dma_start(at[:], src)
            return at

        def big_gemm(ch, at, xsb, xinsb, axf, dsb, with_in, fp8):
            """Feature-major BG chunk: psum tiles [128, 512] per pair + xin."""
            n_ps = 5 if with_in else 4
            ps = [
                pps.tile([P, 512], F32, tag="ps", name=f"ps{i}") for i in range(n_ps)
            ]
            if fp8:
                for ktp in range(KT // 2):
                    st, sp = ktp == 0, ktp == KT // 2 - 1
                    rhs = at[:, 2 * ktp : 2 * ktp + 2, :]
                    for mf in range(4):
                        nc.tensor.matmul(
                            ps[mf][:],
                            lhsT=xsb[:, 2 * ktp : 2 * ktp + 2, ts(mf, P)],
                            rhs=rhs, start=st, stop=sp, perf_mode=DR,
                        )
                    if with_in:
                        nc.tensor.matmul(
                            ps[4][:16],
                            lhsT=xinsb[:, 2 * ktp : 2 * ktp + 2, :],
                            rhs=rhs, start=st, stop=sp, perf_mode=DR,
                        )
            else:
                for kt in range(KT):
                    st, sp = kt == 0, kt == KT - 1
                    rhs = at[:, kt, :]
                    for mf in range(4):
                        nc.tensor.matmul(
                            ps[mf][:],
                            lhsT=xsb[:, kt, ts(mf, P)],
                            rhs=rhs, start=st, stop=sp,
                        )
                    if with_in:
                        nc.tensor.matmul(
                            ps[4][:BL],
                            lhsT=xinsb[:, kt, :],
                            rhs=rhs, start=st, stop=sp,
                        )
            dcol = dsb[:, ds(ch * 512, 512)]
            for mf in range(4):
                nc.vector.tensor_mul(axf[:, ch, mf, :], ps[mf][:], dcol)
            if with_in:
                nc.vector.tensor_mul(
                    axin_sb[:BL, ds(ch * 512, 512)], ps[4][:BL], dcol[:BL]
                )

        # ---------------- GCN1 ----------------
        a1_tiles = [load_a_chunk(ch, fp8_bg1, fine=(ch == 0)) for ch in range(CH)]

        def w1_chunk(ch):
            """Gate-major W1 for chunk ch; r-half -> dram bounce, u -> sbuf."""
            for b in range(BL):
                q = nc.gpsimd
                pw = pps.tile([P, 512], F32, tag="ps", name="pw")
                jo = 64 * (b % 2)
                nc.tensor.matmul(
                    pw[:],
                    lhsT=w1h_sb[jo : jo + 64, :],
                    rhs=axf1_sb[jo : jo + 64, ch, b // 2, :],
                    start=True, stop=False,
                )
                nc.tensor.matmul(
                    pw[:],
                    lhsT=w1i_sb[:, b, :],
                    rhs=axin_sb[:, ds(ch * 512, 512)],
                    start=False, stop=True,
                )
                if ch < 2:
                    sg = spool.tile([P, 512], BF16, tag="sg")
                    nc.scalar.activation(sg[:], pw[:], SIG)
                    q.dma_start(sigr_ds[b].ap()[:, ds(ch * 512, 512)], sg[:])
                    if ch == 1:
                        # coarse xbar transposes: [64,1024] -> [128, 8, 64]
                        for jh in (0, 1):
                            nc.scalar.dma_start_transpose(
                                sigrt_sb[:, ds(8 * jh, 8), b, :],
                                sigr_ds[b].ap()[ds(64 * jh, 64), :],
                            )
                else:
                    nc.scalar.activation(
                        sigu_sb[:, b, ds((ch - 2) * 512, 512)], pw[:], SIG
                    )

        a2_tiles = [None] * CH
        for ch in range(CH):
            big_gemm(ch, a1_tiles[ch], x18_sb if fp8_bg1 else x1_sb,
                     xin_sb, axf1_sb, drep1_sb, True, fp8_bg1)
            w1_chunk(ch)
            if not both_f8:
                if ch == 1:
                    a2_tiles[0] = load_a_chunk(0, fp8_bg2)
                    a2_tiles[1] = load_a_chunk(1, fp8_bg2)
                if ch == 3:
                    a2_tiles[2] = load_a_chunk(2, fp8_bg2)
                    a2_tiles[3] = load_a_chunk(3, fp8_bg2)

        if both_f8:
            a2_tiles = a1_tiles
        # late-needed tensors: load off the critical window
        nc.sync.dma_start(w2h_sb[:], w2h_d.ap())
        nc.sync.dma_start(w2i_sb[:], w2i_d.ap())
        nc.sync.dma_start(ht_sb[:], ht_d.ap())

        # x2 assembly (node-major): x2 = sig_rT * x1, 4 kt per op
        x1src = x18_sb if both_f8 else x1_sb
        for ktg in range(4):
            s3 = sigrt_sb[:, ts(ktg, 4), :, :].rearrange("p k b h -> p (k b h)")
            x1v = x1src[:, ts(ktg, 4), :].rearrange("p k f -> p (k f)")
            x2v = x2_sb[:, ts(ktg, 4), :].rearrange("p k f -> p (k f)")
            nc.vector.tensor_mul(x2v, s3, x1v)

        def w2_chunk(ch):
            for b in range(BL):
                pw = pps.tile([P, 512], F32, tag="ps", name="pw2")
                jo = 64 * (b % 2)
                nc.tensor.matmul(
                    pw[:],
                    lhsT=w2h_sb[jo : jo + 64, :],
                    rhs=axf2_sb[jo : jo + 64, ch, b // 2, :],
                    start=True, stop=False,
                )
                nc.tensor.matmul(
                    pw[:],
                    lhsT=w2i_sb[:, b, :],
                    rhs=axin_sb[:, ds(ch * 512, 512)],
                    start=False, stop=True,
                )
                # de-interleave: c3[64*par+hh, b, t] = c[hh, 2t+par]
                for par in (0, 1):
                    pr = ds(64 * par, 64)
                    nc.scalar.activation(
                        c3_sb[pr, b, ds(256 * ch, 256)],
                        pw[pr, par : par + 511 : 2], TANH,
                    )

        def gate_rng(lo, sz):
            """Gating for t in [lo, lo+sz) over all b, f-major; one out DMA."""
            tsl = ds(lo, sz)
            gt = gpool.tile([P, BL, 512], BF16, tag="g", name="gt")
            for b in range(BL):
                tmpt = gpool.tile([P, 512], BF16, tag="tmp", name="tmpt")
                tmp = tmpt[:, :sz]
                cv = c3_sb[:, b, tsl]
                nc.vector.tensor_sub(tmp[:], ht_sb[:, b, tsl], cv)
                nc.vector.tensor_mul(tmp[:], sigu_sb[:, b, tsl], tmp[:])
                nc.vector.tensor_add(gt[:, b, :sz], tmp[:], cv)
            nc.sync.dma_start(out_d.ap()[:, :, tsl], gt[:, :, :sz])

        for ch in range(CH):
            big_gemm(ch, a2_tiles[ch], x2_sb, None, axf2_sb, drep2_sb,
                     False, fp8_bg2)
            w2_chunk(ch)
            if ch == 2:
                gate_rng(0, 512)      # th0: needs c3 ch0+ch1
                gate_rng(512, 256)    # q2: needs c3 ch2
            elif ch == 3:
                gate_rng(768, 256)    # q3: needs c3 ch3

    nc.finalize()
    return nc


def _prep_inputs(input_tensor, hidden, adj, W1, b1, W2, b2, fp8_bg1, fp8_bg2):
    f32 = np.float32
    bf16 = ml_dtypes.bfloat16
    fp8 = ml_dtypes.float8_e4m3fn
    input_tensor = np.ascontiguousarray(input_tensor, f32)
    hidden = np.ascontiguousarray(hidden, f32)
    adj = np.ascontiguousarray(adj, f32)

    pi = np.concatenate([np.arange(0, N, 2), np.arange(1, N, 2)])
    deg = 1.0 + adj.sum(axis=1, dtype=np.float64)
    d = (deg ** -0.5).astype(f32)
    a_full = (adj + np.eye(N, dtype=f32))[pi]

    any_bf = (not fp8_bg1) or (not fp8_bg2)
    any_f8 = fp8_bg1 or fp8_bg2
    two_dreps = fp8_bg1 != fp8_bg2
    shared = {}
    if any_bf:
        shared["a"] = np.ascontiguousarray(a_full).astype(bf16)
    if any_f8:
        shared["a8"] = np.ascontiguousarray(a_full * S_A).astype(fp8)

    sx1 = S_X if any_f8 else 1.0
    sa1 = S_A if fp8_bg1 else 1.0
    sa2 = S_A if fp8_bg2 else 1.0
    shared["drep1"] = np.ascontiguousarray(
        np.broadcast_to(d / (sa1 * sx1), (P, N)), f32
    )
    if two_dreps:
        shared["drep2"] = np.ascontiguousarray(
            np.broadcast_to(d / (sa2 * sx1), (P, N)), f32
        )

    w1h = np.ascontiguousarray(np.concatenate([W1[1:], W1[1:]], 0).astype(bf16))
    w1i = np.zeros((BL + 1, BL, 2 * H), bf16)
    for bb in range(BL):
        w1i[bb, bb, :] = W1[0].astype(bf16)
        w1i[BL, bb, :] = b1.astype(bf16)
    W2h = W2[1:]
    w2d = np.concatenate([W2h, W2h], 1)          # [64, 128] dup cols
    w2h = np.ascontiguousarray(np.concatenate([w2d, w2d], 0).astype(bf16))
    w2i = np.zeros((BL + 1, BL, 2 * H), bf16)
    for bb in range(BL):
        w2i[bb, bb, :] = np.concatenate([W2[0], W2[0]]).astype(bf16)
        w2i[BL, bb, :] = np.concatenate([b2, b2]).astype(bf16)

    dh = d[None, :, None] * hidden          # (B, N, H)
    din = d[None, :] * input_tensor         # (B, N)

    in_maps = []
    for c in range(NCORES):
        bs = slice(BL * c, BL * c + BL)
        x1f = np.ascontiguousarray(
            dh[bs][:, pi, :].transpose(1, 0, 2).reshape(N, BL * H)
        ) * sx1
        xinf = np.ascontiguousarray(
            din[bs][:, pi].T.reshape(KT, P, BL).transpose(1, 0, 2).reshape(P, KT * BL)
        ) * sx1
        # hidden f-major: ht[64*par+hh, b, t] = hidden[b, 2t+par, hh]
        hv = hidden[bs].reshape(BL, N // 2, 2, H)      # [b, t, par, hh]
        ht = np.ascontiguousarray(
            hv.transpose(2, 3, 0, 1).reshape(P, BL, N // 2)
        ).astype(bf16)
        m = {
            "x1": x1f.astype(bf16), "ht": ht,
            "w1h": w1h, "w1i": w1i, "w2h": w2h, "w2i": w2i,
        }
        m.update(shared)
        if fp8_bg1:
            m["x18"] = x1f.astype(fp8)
            xin8 = np.zeros((P, KT, 16), f32)
            xin8[:, :, :BL] = xinf.reshape(P, KT, BL)
            m["xin8"] = np.ascontiguousarray(xin8.reshape(P, KT * 16)).astype(fp8)
        else:
            m["xin"] = xinf.astype(bf16)
        in_maps.append(m)
    return in_maps


LAST_RESULTS = None


def kernel(input_tensor, hidden, adj, W1, b1, W2, b2):
    global LAST_RESULTS
    key = (FP8_BG1, FP8_BG2)
    if key not in _CACHE:
        _CACHE[key] = _build(*key)
    nc = _CACHE[key]
    in_maps = _prep_inputs(input_tensor, hidden, adj, W1, b1, W2, b2, *key)
    res = run_bass_kernel_spmd(nc, in_maps, core_ids=list(range(NCORES)))
    LAST_RESULTS = res
    outs = []
    for r in res.results:
        g = np.asarray(r["out"], np.float32)           # [128, 8, 1024]
        g = g.reshape(2, H, BL, N // 2)                 # [par, hh, b, t]
        g = g.transpose(2, 3, 0, 1).reshape(BL, N, H)   # [b, 2t+par, hh]
        outs.append(g)
    return np.concatenate(outs, axis=0).astype(np.float32)


if __name__ == "__main__":
    rng = np.random.default_rng(0)
    inputs = {
        "input_tensor": rng.standard_normal((B, N), dtype=np.float32),
        "hidden": rng.standard_normal((B, N, H), dtype=np.float32),
        "adj": rng.random((N, N), dtype=np.float32),
        "W1": rng.standard_normal((H + 1, 2 * H), dtype=np.float32) * 0.15,
        "b1": np.full((2 * H,), 0.4, np.float32),
        "W2": rng.standard_normal((H + 1, H), dtype=np.float32) * 0.15,
        "b2": np.full((H,), 0.6, np.float32),
    }
    out = kernel(**inputs)
    print(out.shape, out.dtype)


# revision 26
# speedup vs baseline: 1.0515x; 1.0515x over previous
"""GCN-GRU cell fused Trainium2 kernel (8-core data parallel), v2.

Math (per batch b):
    A = d * (adj+I).T * d,  d = rowsum(adj+I)^-0.5
    sig   = sigmoid(A @ [input, hidden] @ W1 + b1)   (N, 128)
    r, u  = node-split halves of sig (nodes <1024 / >=1024)
    c     = tanh(A @ [input, r*hidden] @ W2 + b2)
    out   = u * hidden + (1-u) * c

Structure (per core, 8 batches):
  - BG (big GEMM) feature-major: lhsT = X columns (features), rhs = A
    columns (nodes); PSUM out [feat, m], accumulated over 16 k-tiles.
  - W-GEMMs gate-major weight-stationary: lhsT = W (dup-row layout),
    rhs = AX chunk [feat, 512]; out [gates, m] in PSUM -> activation.
  - sig r-half bounced through DRAM + xbar dma transpose -> node-major
    for the x2 = r*x1 assembly (DVE).  Write+transpose share a queue
    (sync for even b, scalar for odd b) so FIFO order guarantees RAW.
  - W2 gates duplicated (cols doubled) so final gating runs f-major on
    DVE with stride-2 column views; out written f-major [f,b,t]; host
    does the final transpose to [b,n,h] (free).
  - Optional fp8e4 DoubleRow big GEMMs (FP8_BG1/FP8_BG2 env flags).
"""

import os
import numpy as np
import ml_dtypes
from contextlib import ExitStack

import concourse.bacc as bacc
import concourse.mybir as mybir
import concourse.tile as tile
from concourse.bass import ts, ds, broadcast_tensor_aps
from concourse.bass_utils import run_bass_kernel_spmd

P = 128
N = 2048
B = 64
H = 64
NCORES = 8
BL = B // NCORES          # 8 batches per core
KT = N // P               # 16 contraction tiles
CH = N // 512             # 4 output chunks of 512
F32 = mybir.dt.float32
BF16 = mybir.dt.bfloat16
FP8 = mybir.dt.float8e4
SIG = mybir.ActivationFunctionType.Sigmoid
TANH = mybir.ActivationFunctionType.Tanh
DR = mybir.MatmulPerfMode.DoubleRow

S_A = 16.0   # fp8 scale on A side
S_X = 32.0   # fp8 scale on X side

FP8_BG1 = os.environ.get("FP8_BG1", "1") == "1"
FP8_BG2 = os.environ.get("FP8_BG2", "1") == "1"

_CACHE = {}


def _build(fp8_bg1: bool, fp8_bg2: bool):
    nc = bacc.Bacc("TRN2", target_bir_lowering=False)

    any_bf = (not fp8_bg1) or (not fp8_bg2)
    any_f8 = fp8_bg1 or fp8_bg2
    two_dreps = fp8_bg1 != fp8_bg2

    # ---- dram tensors ----
    if any_bf:
        a_d = nc.dram_tensor("a", [N, N], BF16, kind="ExternalInput")
    if any_f8:
        a8_d = nc.dram_tensor("a8", [N, N], FP8, kind="ExternalInput")
    x1_d = nc.dram_tensor("x1", [N, BL * H], BF16, kind="ExternalInput")
    if fp8_bg1:
        x18_d = nc.dram_tensor("x18", [N, BL * H], FP8, kind="ExternalInput")
        xin8_d = nc.dram_tensor("xin8", [P, KT * 16], FP8, kind="ExternalInput")
    else:
        xin_d = nc.dram_tensor("xin", [P, KT * BL], BF16, kind="ExternalInput")
    drep1_d = nc.dram_tensor("drep1", [P, N], F32, kind="ExternalInput")
    if two_dreps:
        drep2_d = nc.dram_tensor("drep2", [P, N], F32, kind="ExternalInput")
    ht_d = nc.dram_tensor("ht", [P, BL, N // 2], BF16, kind="ExternalInput")
    w1h_d = nc.dram_tensor("w1h", [2 * H, 2 * H], BF16, kind="ExternalInput")
    w1i_d = nc.dram_tensor("w1i", [BL + 1, BL, 2 * H], BF16, kind="ExternalInput")
    w2h_d = nc.dram_tensor("w2h", [2 * H, 2 * H], BF16, kind="ExternalInput")
    w2i_d = nc.dram_tensor("w2i", [BL + 1, BL, 2 * H], BF16, kind="ExternalInput")
    out_d = nc.dram_tensor("out", [P, BL, N // 2], BF16, kind="ExternalOutput")
    # scratch for the r-half transpose bounce: one tensor per b so
    # whole-tensor DRAM dep tracking doesn't serialize across batches
    sigr_ds = [
        nc.dram_tensor(f"sigr{b}", [P, N // 2], BF16, kind="Internal")
        for b in range(BL)
    ]

    x2dt = FP8 if fp8_bg2 else BF16

    with tile.TileContext(nc) as tc, ExitStack() as ctx:
        const = ctx.enter_context(tc.tile_pool(name="const", bufs=1))
        x1_sb = const.tile([P, KT, BL * H], BF16)
        if fp8_bg1:
            x18_sb = const.tile([P, KT, BL * H], FP8)
            xin_sb = const.tile([P, KT, 16], FP8)
        else:
            xin_sb = const.tile([P, KT, BL], BF16)
        drep1_sb = const.tile([P, N], F32)
        drep2_sb = const.tile([P, N], F32) if two_dreps else drep1_sb
        ht_sb = const.tile([P, BL, N // 2], BF16)
        w1h_sb = const.tile([2 * H, 2 * H], BF16)
        w1i_sb = const.tile([BL + 1, BL, 2 * H], BF16)
        w2h_sb = const.tile([2 * H, 2 * H], BF16)
        w2i_sb = const.tile([BL + 1, BL, 2 * H], BF16)
        axin_sb = const.tile([BL + 1, N], BF16)     # rows 0..7 = d*(A@din), row 8 = ones
        axf1_sb = const.tile([P, CH, 4, 512], BF16)   # [2b x 64feat, ch, pair, m]
        axf2_sb = const.tile([P, CH, 4, 512], x2dt)
        sigu_sb = const.tile([P, BL, N // 2], BF16)    # [gate, b, m-1024]
        sigrt_sb = const.tile([P, KT, BL, H], BF16)    # node-major r gates
        x2_sb = const.tile([P, KT, BL * H], x2dt)
        c3_sb = const.tile([P, BL, N // 2], BF16)      # de-interleaved c, [f, b, t]

        x1_r = x1_d.ap().rearrange("(kt p) f -> p kt f", p=P)
        if fp8_bg1:
            x18_r = x18_d.ap().rearrange("(kt p) f -> p kt f", p=P)
            for g in range(8):
                nc.scalar.dma_start(x18_sb[:, ts(g, 2), :], x18_r[:, ts(g, 2), :])
            nc.scalar.dma_start(
                xin_sb[:], xin8_d.ap().rearrange("p (kt b) -> p kt b", b=16)
            )
        else:
            nc.scalar.dma_start(
                xin_sb[:], xin_d.ap().rearrange("p (kt b) -> p kt b", b=BL)
            )
        nc.scalar.dma_start(drep1_sb[:], drep1_d.ap())
        if two_dreps:
            nc.scalar.dma_start(drep2_sb[:], drep2_d.ap())
        nc.scalar.dma_start(w1h_sb[:], w1h_d.ap())
        nc.scalar.dma_start(w1i_sb[:], w1i_d.ap())
        nc.vector.memset(axin_sb[:], 1.0)

        both_f8 = fp8_bg1 and fp8_bg2
        if both_f8:
            a8_sb = const.tile([P, KT, N], FP8)
        apool = ctx.enter_context(tc.tile_pool(name="ap", bufs=3))
        spool = ctx.enter_context(tc.tile_pool(name="sp", bufs=4))
        gpool = ctx.enter_context(tc.tile_pool(name="gp", bufs=2))
        pps = ctx.enter_context(tc.tile_pool(name="ps", bufs=8, space="PSUM"))

        if any_bf:
            a_r = a_d.ap().rearrange("(kt p) m -> p kt m", p=P)
        if any_f8:
            a8_r = a8_d.ap().rearrange("(kt p) m -> p kt m", p=P)

        def load_a_chunk(ch, fp8, fine=False):
            if both_f8:
                at = a8_sb[:, :, ds(ch * 512, 512)]
                src = a8_r[:, :, ds(ch * 512, 512)]
                if fine:
                    for g in range(4):
                        nc.sync.dma_start(at[:, ts(g, 4), :], src[:, ts(g, 4), :])
                else:
                    nc.sync.dma_start(at[:], src)
                return at
            dt = FP8 if fp8 else BF16
            at = apool.tile([P, KT, 512], dt, tag="a8" if fp8 else "abf")
            src = (a8_r if fp8 else a_r)[:, :, ds(ch * 512, 512)]
            if fine:
                for g in range(4):
                    nc.sync.dma_start(at[:, ts(g, 4), :], src[:, ts(g, 4), :])
            else:
                nc.sync.dma_start(at[:], src)
            return at

        def big_gemm(ch, at, xsb, xinsb, axf, dsb, with_in, fp8):
            """Feature-major BG chunk: psum tiles [128, 512] per pair + xin."""
            n_ps = 5 if with_in else 4
            ps = [
                pps.tile([P, 512], F32, tag="ps", name=f"ps{i}") for i in range(n_ps)
            ]
            if fp8:
                for ktp in range(KT // 2):
                    st, sp = ktp == 0, ktp == KT // 2 - 1
                    rhs = at[:, 2 * ktp : 2 * ktp + 2, :]
                    for mf in range(4):
                        nc.tensor.matmul(
                            ps[mf][:],
                            lhsT=xsb[:, 2 * ktp : 2 * ktp + 2, ts(mf, P)],
                            rhs=rhs, start=st, stop=sp, perf_mode=DR,
                        )
                    if with_in:
                        nc.tensor.matmul(
                            ps[4][:16],
                            lhsT=xinsb[:, 2 * ktp : 2 * ktp + 2, :],
                            rhs=rhs, start=st, stop=sp, perf_mode=DR,
                        )
            else:
                for kt in range(KT):
                    st, sp = kt == 0, kt == KT - 1
                    rhs = at[:, kt, :]
                    for mf in range(4):
                        nc.tensor.matmul(
                            ps[mf][:],
                            lhsT=xsb[:, kt, ts(mf, P)],
                            rhs=rhs, start=st, stop=sp,
                        )
                    if with_in:
                        nc.tensor.matmul(
                            ps[4][:BL],
                            lhsT=xinsb[:, kt, :],
                            rhs=rhs, start=st, stop=sp,
                        )
            dcol = dsb[:, ds(ch * 512, 512)]
            for mf in range(4):
                nc.vector.tensor_mul(axf[:, ch, mf, :], ps[mf][:], dcol)
            if with_in:
                nc.vector.tensor_mul(
                    axin_sb[:BL, ds(ch * 512, 512)], ps[4][:BL], dcol[:BL]
                )

        # ---------------- GCN1 ----------------
        a1_tiles = [load_a_chunk(ch, fp8_bg1, fine=(ch == 0)) for ch in range(CH)]
        for g in range(4):
            nc.sync.dma_start(x1_sb[:, ts(g, 4), :], x1_r[:, ts(g, 4), :])

        def w1_chunk(ch):
            """Gate-major W1 for chunk ch; r-half -> dram bounce, u -> sbuf."""
            for b in range(BL):
                q = nc.gpsimd
                pw = pps.tile([P, 512], F32, tag="ps", name="pw")
                jo = 64 * (b % 2)
                nc.tensor.matmul(
                    pw[:],
                    lhsT=w1h_sb[jo : jo + 64, :],
                    rhs=axf1_sb[jo : jo + 64, ch, b // 2, :],
                    start=True, stop=False,
                )
                nc.tensor.matmul(
                    pw[:],
                    lhsT=w1i_sb[:, b, :],
                    rhs=axin_sb[:, ds(ch * 512, 512)],
                    start=False, stop=True,
                )
                if ch < 2:
                    sg = spool.tile([P, 512], BF16, tag="sg")
                    nc.scalar.activation(sg[:], pw[:], SIG)
                    q.dma_start(sigr_ds[b].ap()[:, ds(ch * 512, 512)], sg[:])
                    if ch == 1:
                        # coarse xbar transposes: [64,1024] -> [128, 8, 64]
                        for jh in (0, 1):
                            nc.scalar.dma_start_transpose(
                                sigrt_sb[:, ds(8 * jh, 8), b, :],
                                sigr_ds[b].ap()[ds(64 * jh, 64), :],
                            )
                else:
                    nc.scalar.activation(
                        sigu_sb[:, b, ds((ch - 2) * 512, 512)], pw[:], SIG
                    )

        a2_tiles = [None] * CH
        for ch in range(CH):
            big_gemm(ch, a1_tiles[ch], x18_sb if fp8_bg1 else x1_sb,
                     xin_sb, axf1_sb, drep1_sb, True, fp8_bg1)
            w1_chunk(ch)
            if not both_f8:
                if ch == 1:
                    a2_tiles[0] = load_a_chunk(0, fp8_bg2)
                    a2_tiles[1] = load_a_chunk(1, fp8_bg2)
                if ch == 3:
                    a2_tiles[2] = load_a_chunk(2, fp8_bg2)
                    a2_tiles[3] = load_a_chunk(3, fp8_bg2)

        if both_f8:
            a2_tiles = a1_tiles
        # late-needed tensors: load off the critical window
        nc.sync.dma_start(w2h_sb[:], w2h_d.ap())
        nc.sync.dma_start(w2i_sb[:], w2i_d.ap())
        nc.sync.dma_start(ht_sb[:], ht_d.ap())

        # x2 assembly (node-major): x2 = sig_rT * x1, 4 kt per op
        for ktg in range(4):
            s3 = sigrt_sb[:, ts(ktg, 4), :, :].rearrange("p k b h -> p (k b h)")
            x1v = x1_sb[:, ts(ktg, 4), :].rearrange("p k f -> p (k f)")
            x2v = x2_sb[:, ts(ktg, 4), :].rearrange("p k f -> p (k f)")
            nc.vector.tensor_mul(x2v, s3, x1v)

        def w2_chunk(ch):
            for b in range(BL):
                pw = pps.tile([P, 512], F32, tag="ps", name="pw2")
                jo = 64 * (b % 2)
                nc.tensor.matmul(
                    pw[:],
                    lhsT=w2h_sb[jo : jo + 64, :],
                    rhs=axf2_sb[jo : jo + 64, ch, b // 2, :],
                    start=True, stop=False,
                )
                nc.tensor.matmul(
                    pw[:],
                    lhsT=w2i_sb[:, b, :],
                    rhs=axin_sb[:, ds(ch * 512, 512)],
                    start=False, stop=True,
                )
                # de-interleave: c3[64*par+hh, b, t] = c[hh, 2t+par]
                for par in (0, 1):
                    pr = ds(64 * par, 64)
                    nc.scalar.activation(
                        c3_sb[pr, b, ds(256 * ch, 256)],
                        pw[pr, par : par + 511 : 2], TANH,
                    )

        def gate_rng(lo, sz):
            """Gating for t in [lo, lo+sz) over all b, f-major; one out DMA."""
            tsl = ds(lo, sz)
            gt = gpool.tile([P, BL, 512], BF16, tag="g", name="gt")
            for b in range(BL):
                tmpt = gpool.tile([P, 512], BF16, tag="tmp", name="tmpt")
                tmp = tmpt[:, :sz]
                cv = c3_sb[:, b, tsl]
                nc.vector.tensor_sub(tmp[:], ht_sb[:, b, tsl], cv)
                nc.vector.tensor_mul(tmp[:], sigu_sb[:, b, tsl], tmp[:])
                nc.vector.tensor_add(gt[:, b, :sz], tmp[:], cv)
            nc.sync.dma_start(out_d.ap()[:, :, tsl], gt[:, :, :sz])

        for ch in range(CH):
            big_gemm(ch, a2_tiles[ch], x2_sb, None, axf2_sb, drep2_sb,
                     False, fp8_bg2)
            w2_chunk(ch)
            if ch == 2:
                gate_rng(0, 512)      # th0: needs c3 ch0+ch1
                gate_rng(512, 256)    # q2: needs c3 ch2
            elif ch == 3:
                gate_rng(768, 256)    # q3: needs c3 ch3

    nc.finalize()
    return nc


def _prep_inputs(input_tensor, hidden, adj, W1, b1, W2, b2, fp8_bg1, fp8_bg2):
    f32 = np.float32
    bf16 = ml_dtypes.bfloat16
    fp8 = ml_dtypes.float8_e4m3fn
    input_tensor = np.ascontiguousarray(input_tensor, f32)
    hidden = np.ascontiguousarray(hidden, f32)
    adj = np.ascontiguousarray(adj, f32)

    pi = np.concatenate([np.arange(0, N, 2), np.arange(1, N, 2)])
    deg = 1.0 + adj.sum(axis=1, dtype=np.float64)
    d = (deg ** -0.5).astype(f32)
    a_full = (adj + np.eye(N, dtype=f32))[pi]

    any_bf = (not fp8_bg1) or (not fp8_bg2)
    any_f8 = fp8_bg1 or fp8_bg2
    two_dreps = fp8_bg1 != fp8_bg2
    shared = {}
    if any_bf:
        shared["a"] = np.ascontiguousarray(a_full).astype(bf16)
    if any_f8:
        shared["a8"] = np.ascontiguousarray(a_full * S_A).astype(fp8)

    sx1 = S_X if any_f8 else 1.0
    sa1 = S_A if fp8_bg1 else 1.0
    sa2 = S_A if fp8_bg2 else 1.0
    shared["drep1"] = np.ascontiguousarray(
        np.broadcast_to(d / (sa1 * sx1), (P, N)), f32
    )
    if two_dreps:
        shared["drep2"] = np.ascontiguousarray(
            np.broadcast_to(d / (sa2 * sx1), (P, N)), f32
        )

    w1h = np.ascontiguousarray(np.concatenate([W1[1:], W1[1:]], 0).astype(bf16))
    w1i = np.zeros((BL + 1, BL, 2 * H), bf16)
    for bb in range(BL):
        w1i[bb, bb, :] = W1[0].astype(bf16)
        w1i[BL, bb, :] = b1.astype(bf16)
    W2h = W2[1:]
    w2d = np.concatenate([W2h, W2h], 1)          # [64, 128] dup cols
    w2h = np.ascontiguousarray(np.concatenate([w2d, w2d], 0).astype(bf16))
    w2i = np.zeros((BL + 1, BL, 2 * H), bf16)
    for bb in range(BL):
        w2i[bb, bb, :] = np.concatenate([W2[0], W2[0]]).astype(bf16)
        w2i[BL, bb, :] = np.concatenate([b2, b2]).astype(bf16)

    dh = d[None, :, None] * hidden          # (B, N, H)
    din = d[None, :] * input_tensor         # (B, N)

    in_maps = []
    for c in range(NCORES):
        bs = slice(BL * c, BL * c + BL)
        x1f = np.ascontiguousarray(
            dh[bs][:, pi, :].transpose(1, 0, 2).reshape(N, BL * H)
        ) * sx1
        xinf = np.ascontiguousarray(
            din[bs][:, pi].T.reshape(KT, P, BL).transpose(1, 0, 2).reshape(P, KT * BL)
        ) * sx1
        # hidden f-major: ht[64*par+hh, b, t] = hidden[b, 2t+par, hh]
        hv = hidden[bs].reshape(BL, N // 2, 2, H)      # [b, t, par, hh]
        ht = np.ascontiguousarray(
            hv.transpose(2, 3, 0, 1).reshape(P, BL, N // 2)
        ).astype(bf16)
        m = {
            "x1": x1f.astype(bf16), "ht": ht,
            "w1h": w1h, "w1i": w1i, "w2h": w2h, "w2i": w2i,
        }
        m.update(shared)
        if fp8_bg1:
            m["x18"] = x1f.astype(fp8)
            xin8 = np.zeros((P, KT, 16), f32)
            xin8[:, :, :BL] = xinf.reshape(P, KT, BL)
            m["xin8"] = np.ascontiguousarray(xin8.reshape(P, KT * 16)).astype(fp8)
        else:
            m["xin"] = xinf.astype(bf16)
        in_maps.append(m)
    return in_maps


LAST_RESULTS = None


def kernel(input_tensor, hidden, adj, W1, b1, W2, b2):
    global LAST_RESULTS
    key = (FP8_BG1, FP8_BG2)
    if key not in _CACHE:
        _CACHE[key] = _build(*key)
    nc = _CACHE[key]
    in_maps = _prep_inputs(input_tensor, hidden, adj, W1, b1, W2, b2, *key)
    res = run_bass_kernel_spmd(nc, in_maps, core_ids=list(range(NCORES)))
    LAST_RESULTS = res
    outs = []
    for r in res.results:
        g = np.asarray(r["out"], np.float32)           # [128, 8, 1024]
        g = g.reshape(2, H, BL, N // 2)                 # [par, hh, b, t]
        g = g.transpose(2, 3, 0, 1).reshape(BL, N, H)   # [b, 2t+par, hh]
        outs.append(g)
    return np.concatenate(outs, axis=0).astype(np.float32)


if __name__ == "__main__":
    rng = np.random.default_rng(0)
    inputs = {
        "input_tensor": rng.standard_normal((B, N), dtype=np.float32),
        "hidden": rng.standard_normal((B, N, H), dtype=np.float32),
        "adj": rng.random((N, N), dtype=np.float32),
        "W1": rng.standard_normal((H + 1, 2 * H), dtype=np.float32) * 0.15,
        "b1": np.full((2 * H,), 0.4, np.float32),
        "W2": rng.standard_normal((H + 1, H), dtype=np.float32) * 0.15,
        "b2": np.full((H,), 0.6, np.float32),
    }
    out = kernel(**inputs)
    print(out.shape, out.dtype)


# revision 27
# speedup vs baseline: 1.0861x; 1.0329x over previous
"""GCN-GRU cell fused Trainium2 kernel (8-core data parallel), v2.

Math (per batch b):
    A = d * (adj+I).T * d,  d = rowsum(adj+I)^-0.5
    sig   = sigmoid(A @ [input, hidden] @ W1 + b1)   (N, 128)
    r, u  = node-split halves of sig (nodes <1024 / >=1024)
    c     = tanh(A @ [input, r*hidden] @ W2 + b2)
    out   = u * hidden + (1-u) * c

Structure (per core, 8 batches):
  - BG (big GEMM) feature-major: lhsT = X columns (features), rhs = A
    columns (nodes); PSUM out [feat, m], accumulated over 16 k-tiles.
  - W-GEMMs gate-major weight-stationary: lhsT = W (dup-row layout),
    rhs = AX chunk [feat, 512]; out [gates, m] in PSUM -> activation.
  - sig r-half bounced through DRAM + xbar dma transpose -> node-major
    for the x2 = r*x1 assembly (DVE).  Write+transpose share a queue
    (sync for even b, scalar for odd b) so FIFO order guarantees RAW.
  - W2 gates duplicated (cols doubled) so final gating runs f-major on
    DVE with stride-2 column views; out written f-major [f,b,t]; host
    does the final transpose to [b,n,h] (free).
  - Optional fp8e4 DoubleRow big GEMMs (FP8_BG1/FP8_BG2 env flags).
"""

import os
import numpy as np
import ml_dtypes
from contextlib import ExitStack

import concourse.bacc as bacc
import concourse.mybir as mybir
import concourse.tile as tile
from concourse.bass import ts, ds, broadcast_tensor_aps
from concourse.bass_utils import run_bass_kernel_spmd

P = 128
N = 2048
B = 64
H = 64
NCORES = 8
BL = B // NCORES          # 8 batches per core
KT = N // P               # 16 contraction tiles
CH = N // 512             # 4 output chunks of 512
F32 = mybir.dt.float32
BF16 = mybir.dt.bfloat16
FP8 = mybir.dt.float8e4
SIG = mybir.ActivationFunctionType.Sigmoid
TANH = mybir.ActivationFunctionType.Tanh
DR = mybir.MatmulPerfMode.DoubleRow

S_A = 16.0   # fp8 scale on A side
S_X = 32.0   # fp8 scale on X side

FP8_BG1 = os.environ.get("FP8_BG1", "1") == "1"
FP8_BG2 = os.environ.get("FP8_BG2", "1") == "1"

_CACHE = {}


def _build(fp8_bg1: bool, fp8_bg2: bool):
    nc = bacc.Bacc("TRN2", target_bir_lowering=False)

    any_bf = (not fp8_bg1) or (not fp8_bg2)
    any_f8 = fp8_bg1 or fp8_bg2
    two_dreps = fp8_bg1 != fp8_bg2

    # ---- dram tensors ----
    if any_bf:
        a_d = nc.dram_tensor("a", [N, N], BF16, kind="ExternalInput")
    if any_f8:
        a8_d = nc.dram_tensor("a8", [N, N], FP8, kind="ExternalInput")
    x1_d = nc.dram_tensor("x1", [N, BL * H], BF16, kind="ExternalInput")
    if fp8_bg1:
        x18_d = nc.dram_tensor("x18", [N, BL * H], FP8, kind="ExternalInput")
        xin8_d = nc.dram_tensor("xin8", [P, KT * 16], FP8, kind="ExternalInput")
    else:
        xin_d = nc.dram_tensor("xin", [P, KT * BL], BF16, kind="ExternalInput")
    drep1_d = nc.dram_tensor("drep1", [P, N], F32, kind="ExternalInput")
    if two_dreps:
        drep2_d = nc.dram_tensor("drep2", [P, N], F32, kind="ExternalInput")
    ht_d = nc.dram_tensor("ht", [P, BL, N // 2], BF16, kind="ExternalInput")
    w1h_d = nc.dram_tensor("w1h", [2 * H, 2 * H], BF16, kind="ExternalInput")
    w1i_d = nc.dram_tensor("w1i", [BL + 1, BL, 2 * H], BF16, kind="ExternalInput")
    w2h_d = nc.dram_tensor("w2h", [2 * H, 2 * H], BF16, kind="ExternalInput")
    w2i_d = nc.dram_tensor("w2i", [BL + 1, BL, 2 * H], BF16, kind="ExternalInput")
    out_d = nc.dram_tensor("out", [P, BL, N // 2], BF16, kind="ExternalOutput")
    # scratch for the r-half transpose bounce: one tensor per b so
    # whole-tensor DRAM dep tracking doesn't serialize across batches
    sigr_ds = [
        nc.dram_tensor(f"sigr{b}", [P, N // 2], BF16, kind="Internal")
        for b in range(BL)
    ]

    x2dt = FP8 if fp8_bg2 else BF16

    with tile.TileContext(nc) as tc, ExitStack() as ctx:
        const = ctx.enter_context(tc.tile_pool(name="const", bufs=1))
        x1_sb = const.tile([P, KT, BL * H], BF16)
        if fp8_bg1:
            x18_sb = const.tile([P, KT, BL * H], FP8)
            xin_sb = const.tile([P, KT, 16], FP8)
        else:
            xin_sb = const.tile([P, KT, BL], BF16)
        drep1_sb = const.tile([P, N], F32)
        drep2_sb = const.tile([P, N], F32) if two_dreps else drep1_sb
        ht_sb = const.tile([P, BL, N // 2], BF16)
        w1h_sb = const.tile([2 * H, 2 * H], BF16)
        w1i_sb = const.tile([BL + 1, BL, 2 * H], BF16)
        w2h_sb = const.tile([2 * H, 2 * H], BF16)
        w2i_sb = const.tile([BL + 1, BL, 2 * H], BF16)
        axin_sb = const.tile([BL + 1, N], BF16)     # rows 0..7 = d*(A@din), row 8 = ones
        axf1_sb = const.tile([P, CH, 4, 512], BF16)   # [2b x 64feat, ch, pair, m]
        axf2_sb = const.tile([P, CH, 4, 512], x2dt)
        sigu_sb = const.tile([P, BL, N // 2], BF16)    # [gate, b, m-1024]
        sigrt_sb = const.tile([P, KT, BL, H], BF16)    # node-major r gates
        x2_sb = const.tile([P, KT, BL * H], x2dt)
        c3_sb = const.tile([P, BL, N // 2], BF16)      # de-interleaved c, [f, b, t]

        x1_r = x1_d.ap().rearrange("(kt p) f -> p kt f", p=P)
        if fp8_bg1:
            x18_r = x18_d.ap().rearrange("(kt p) f -> p kt f", p=P)
            for g in range(8):
                nc.scalar.dma_start(x18_sb[:, ts(g, 2), :], x18_r[:, ts(g, 2), :])
            nc.scalar.dma_start(
                xin_sb[:], xin8_d.ap().rearrange("p (kt b) -> p kt b", b=16)
            )
        else:
            nc.scalar.dma_start(
                xin_sb[:], xin_d.ap().rearrange("p (kt b) -> p kt b", b=BL)
            )
        nc.scalar.dma_start(drep1_sb[:], drep1_d.ap())
        if two_dreps:
            nc.scalar.dma_start(drep2_sb[:], drep2_d.ap())
        nc.scalar.dma_start(w1h_sb[:], w1h_d.ap())
        nc.scalar.dma_start(w1i_sb[:], w1i_d.ap())
        nc.vector.memset(axin_sb[:], 1.0)
        for g in range(4):
            nc.scalar.dma_start(x1_sb[:, ts(g, 4), :], x1_r[:, ts(g, 4), :])

        both_f8 = fp8_bg1 and fp8_bg2
        if both_f8:
            a8_sb = const.tile([P, KT, N], FP8)
        apool = ctx.enter_context(tc.tile_pool(name="ap", bufs=3))
        spool = ctx.enter_context(tc.tile_pool(name="sp", bufs=4))
        gpool = ctx.enter_context(tc.tile_pool(name="gp", bufs=2))
        pps = ctx.enter_context(tc.tile_pool(name="ps", bufs=8, space="PSUM"))

        if any_bf:
            a_r = a_d.ap().rearrange("(kt p) m -> p kt m", p=P)
        if any_f8:
            a8_r = a8_d.ap().rearrange("(kt p) m -> p kt m", p=P)

        def load_a_chunk(ch, fp8, fine=False):
            if both_f8:
                at = a8_sb[:, :, ds(ch * 512, 512)]
                src = a8_r[:, :, ds(ch * 512, 512)]
                if fine:
                    for g in range(4):
                        nc.sync.dma_start(at[:, ts(g, 4), :], src[:, ts(g, 4), :])
                else:
                    nc.sync.dma_start(at[:], src)
                return at
            dt = FP8 if fp8 else BF16
            at = apool.tile([P, KT, 512], dt, tag="a8" if fp8 else "abf")
            src = (a8_r if fp8 else a_r)[:, :, ds(ch * 512, 512)]
            if fine:
                for g in range(4):
                    nc.sync.dma_start(at[:, ts(g, 4), :], src[:, ts(g, 4), :])
            else:
                nc.sync.dma_start(at[:], src)
            return at

        def big_gemm(ch, at, xsb, xinsb, axf, dsb, with_in, fp8):
            """Feature-major BG chunk: psum tiles [128, 512] per pair + xin."""
            n_ps = 5 if with_in else 4
            ps = [
                pps.tile([P, 512], F32, tag="ps", name=f"ps{i}") for i in range(n_ps)
            ]
            if fp8:
                for ktp in range(KT // 2):
                    st, sp = ktp == 0, ktp == KT // 2 - 1
                    rhs = at[:, 2 * ktp : 2 * ktp + 2, :]
                    for mf in range(4):
                        nc.tensor.matmul(
                            ps[mf][:],
                            lhsT=xsb[:, 2 * ktp : 2 * ktp + 2, ts(mf, P)],
                            rhs=rhs, start=st, stop=sp, perf_mode=DR,
                        )
                    if with_in:
                        nc.tensor.matmul(
                            ps[4][:16],
                            lhsT=xinsb[:, 2 * ktp : 2 * ktp + 2, :],
                            rhs=rhs, start=st, stop=sp, perf_mode=DR,
                        )
            else:
                for kt in range(KT):
                    st, sp = kt == 0, kt == KT - 1
                    rhs = at[:, kt, :]
                    for mf in range(4):
                        nc.tensor.matmul(
                            ps[mf][:],
                            lhsT=xsb[:, kt, ts(mf, P)],
                            rhs=rhs, start=st, stop=sp,
                        )
                    if with_in:
                        nc.tensor.matmul(
                            ps[4][:BL],
                            lhsT=xinsb[:, kt, :],
                            rhs=rhs, start=st, stop=sp,
                        )
            dcol = dsb[:, ds(ch * 512, 512)]
            for mf in range(4):
                nc.vector.tensor_mul(axf[:, ch, mf, :], ps[mf][:], dcol)
            if with_in:
                nc.vector.tensor_mul(
                    axin_sb[:BL, ds(ch * 512, 512)], ps[4][:BL], dcol[:BL]
                )

        # ---------------- GCN1 ----------------
        a1_tiles = [load_a_chunk(ch, fp8_bg1, fine=(ch == 0)) for ch in range(CH)]

        def w1_chunk(ch):
            """Gate-major W1 for chunk ch; r-half -> dram bounce, u -> sbuf."""
            for b in range(BL):
                q = nc.gpsimd
                pw = pps.tile([P, 512], F32, tag="ps", name="pw")
                jo = 64 * (b % 2)
                nc.tensor.matmul(
                    pw[:],
                    lhsT=w1h_sb[jo : jo + 64, :],
                    rhs=axf1_sb[jo : jo + 64, ch, b // 2, :],
                    start=True, stop=False,
                )
                nc.tensor.matmul(
                    pw[:],
                    lhsT=w1i_sb[:, b, :],
                    rhs=axin_sb[:, ds(ch * 512, 512)],
                    start=False, stop=True,
                )
                if ch < 2:
                    sg = spool.tile([P, 512], BF16, tag="sg")
                    nc.scalar.activation(sg[:], pw[:], SIG)
                    q.dma_start(sigr_ds[b].ap()[:, ds(ch * 512, 512)], sg[:])
                    if ch == 1:
                        # coarse xbar transposes: [64,1024] -> [128, 8, 64]
                        for jh in (0, 1):
                            nc.scalar.dma_start_transpose(
                                sigrt_sb[:, ds(8 * jh, 8), b, :],
                                sigr_ds[b].ap()[ds(64 * jh, 64), :],
                            )
                else:
                    nc.scalar.activation(
                        sigu_sb[:, b, ds((ch - 2) * 512, 512)], pw[:], SIG
                    )

        a2_tiles = [None] * CH
        for ch in range(CH):
            big_gemm(ch, a1_tiles[ch], x18_sb if fp8_bg1 else x1_sb,
                     xin_sb, axf1_sb, drep1_sb, True, fp8_bg1)
            w1_chunk(ch)
            if not both_f8:
                if ch == 1:
                    a2_tiles[0] = load_a_chunk(0, fp8_bg2)
                    a2_tiles[1] = load_a_chunk(1, fp8_bg2)
                if ch == 3:
                    a2_tiles[2] = load_a_chunk(2, fp8_bg2)
                    a2_tiles[3] = load_a_chunk(3, fp8_bg2)

        if both_f8:
            a2_tiles = a1_tiles
        # late-needed tensors: load off the critical window
        nc.sync.dma_start(w2h_sb[:], w2h_d.ap())
        nc.sync.dma_start(w2i_sb[:], w2i_d.ap())
        nc.sync.dma_start(ht_sb[:], ht_d.ap())

        # x2 assembly (node-major): x2 = sig_rT * x1, 4 kt per op
        for ktg in range(4):
            s3 = sigrt_sb[:, ts(ktg, 4), :, :].rearrange("p k b h -> p (k b h)")
            x1v = x1_sb[:, ts(ktg, 4), :].rearrange("p k f -> p (k f)")
            x2v = x2_sb[:, ts(ktg, 4), :].rearrange("p k f -> p (k f)")
            nc.vector.tensor_mul(x2v, s3, x1v)

        def w2_chunk(ch):
            for b in range(BL):
                pw = pps.tile([P, 512], F32, tag="ps", name="pw2")
                jo = 64 * (b % 2)
                nc.tensor.matmul(
                    pw[:],
                    lhsT=w2h_sb[jo : jo + 64, :],
                    rhs=axf2_sb[jo : jo + 64, ch, b // 2, :],
                    start=True, stop=False,
                )
                nc.tensor.matmul(
                    pw[:],
                    lhsT=w2i_sb[:, b, :],
                    rhs=axin_sb[:, ds(ch * 512, 512)],
                    start=False, stop=True,
                )
                # de-interleave: c3[64*par+hh, b, t] = c[hh, 2t+par]
                for par in (0, 1):
                    pr = ds(64 * par, 64)
                    nc.scalar.activation(
                        c3_sb[pr, b, ds(256 * ch, 256)],
                        pw[pr, par : par + 511 : 2], TANH,
                    )

        def gate_rng(lo, sz):
            """Gating for t in [lo, lo+sz) over all b, f-major; one out DMA."""
            tsl = ds(lo, sz)
            gt = gpool.tile([P, BL, 512], BF16, tag="g", name="gt")
            for b in range(BL):
                tmpt = gpool.tile([P, 512], BF16, tag="tmp", name="tmpt")
                tmp = tmpt[:, :sz]
                cv = c3_sb[:, b, tsl]
                nc.vector.tensor_sub(tmp[:], ht_sb[:, b, tsl], cv)
                nc.vector.tensor_mul(tmp[:], sigu_sb[:, b, tsl], tmp[:])
                nc.vector.tensor_add(gt[:, b, :sz], tmp[:], cv)
            nc.sync.dma_start(out_d.ap()[:, :, tsl], gt[:, :, :sz])

        for ch in range(CH):
            big_gemm(ch, a2_tiles[ch], x2_sb, None, axf2_sb, drep2_sb,
                     False, fp8_bg2)
            w2_chunk(ch)
            if ch == 2:
                gate_rng(0, 512)      # th0: needs c3 ch0+ch1
                gate_rng(512, 256)    # q2: needs c3 ch2
            elif ch == 3:
                gate_rng(768, 256)    # q3: needs c3 ch3

    nc.finalize()
    return nc


def _prep_inputs(input_tensor, hidden, adj, W1, b1, W2, b2, fp8_bg1, fp8_bg2):
    f32 = np.float32
    bf16 = ml_dtypes.bfloat16
    fp8 = ml_dtypes.float8_e4m3fn
    input_tensor = np.ascontiguousarray(input_tensor, f32)
    hidden = np.ascontiguousarray(hidden, f32)
    adj = np.ascontiguousarray(adj, f32)

    pi = np.concatenate([np.arange(0, N, 2), np.arange(1, N, 2)])
    deg = 1.0 + adj.sum(axis=1, dtype=np.float64)
    d = (deg ** -0.5).astype(f32)
    a_full = (adj + np.eye(N, dtype=f32))[pi]

    any_bf = (not fp8_bg1) or (not fp8_bg2)
    any_f8 = fp8_bg1 or fp8_bg2
    two_dreps = fp8_bg1 != fp8_bg2
    shared = {}
    if any_bf:
        shared["a"] = np.ascontiguousarray(a_full).astype(bf16)
    if any_f8:
        shared["a8"] = np.ascontiguousarray(a_full * S_A).astype(fp8)

    sx1 = S_X if any_f8 else 1.0
    sa1 = S_A if fp8_bg1 else 1.0
    sa2 = S_A if fp8_bg2 else 1.0
    shared["drep1"] = np.ascontiguousarray(
        np.broadcast_to(d / (sa1 * sx1), (P, N)), f32
    )
    if two_dreps:
        shared["drep2"] = np.ascontiguousarray(
            np.broadcast_to(d / (sa2 * sx1), (P, N)), f32
        )

    w1h = np.ascontiguousarray(np.concatenate([W1[1:], W1[1:]], 0).astype(bf16))
    w1i = np.zeros((BL + 1, BL, 2 * H), bf16)
    for bb in range(BL):
        w1i[bb, bb, :] = W1[0].astype(bf16)
        w1i[BL, bb, :] = b1.astype(bf16)
    W2h = W2[1:]
    w2d = np.concatenate([W2h, W2h], 1)          # [64, 128] dup cols
    w2h = np.ascontiguousarray(np.concatenate([w2d, w2d], 0).astype(bf16))
    w2i = np.zeros((BL + 1, BL, 2 * H), bf16)
    for bb in range(BL):
        w2i[bb, bb, :] = np.concatenate([W2[0], W2[0]]).astype(bf16)
        w2i[BL, bb, :] = np.concatenate([b2, b2]).astype(bf16)

    dh = d[None, :, None] * hidden          # (B, N, H)
    din = d[None, :] * input_tensor         # (B, N)

    in_maps = []
    for c in range(NCORES):
        bs = slice(BL * c, BL * c + BL)
        x1f = np.ascontiguousarray(
            dh[bs][:, pi, :].transpose(1, 0, 2).reshape(N, BL * H)
        ) * sx1
        xinf = np.ascontiguousarray(
            din[bs][:, pi].T.reshape(KT, P, BL).transpose(1, 0, 2).reshape(P, KT * BL)
        ) * sx1
        # hidden f-major: ht[64*par+hh, b, t] = hidden[b, 2t+par, hh]
        hv = hidden[bs].reshape(BL, N // 2, 2, H)      # [b, t, par, hh]
        ht = np.ascontiguousarray(
            hv.transpose(2, 3, 0, 1).reshape(P, BL, N // 2)
        ).astype(bf16)
        m = {
            "x1": x1f.astype(bf16), "ht": ht,
            "w1h": w1h, "w1i": w1i, "w2h": w2h, "w2i": w2i,
        }
        m.update(shared)
        if fp8_bg1:
            m["x18"] = x1f.astype(fp8)
            xin8 = np.zeros((P, KT, 16), f32)
            xin8[:, :, :BL] = xinf.reshape(P, KT, BL)
            m["xin8"] = np.ascontiguousarray(xin8.reshape(P, KT * 16)).astype(fp8)
        else:
            m["xin"] = xinf.astype(bf16)
        in_maps.append(m)
    return in_maps


LAST_RESULTS = None


def kernel(input_tensor, hidden, adj, W1, b1, W2, b2):
    global LAST_RESULTS
    key = (FP8_BG1, FP8_BG2)
    if key not in _CACHE:
        _CACHE[key] = _build(*key)
    nc = _CACHE[key]
    in_maps = _prep_inputs(input_tensor, hidden, adj, W1, b1, W2, b2, *key)
    res = run_bass_kernel_spmd(nc, in_maps, core_ids=list(range(NCORES)))
    LAST_RESULTS = res
    outs = []
    for r in res.results:
        g = np.asarray(r["out"], np.float32)           # [128, 8, 1024]
        g = g.reshape(2, H, BL, N // 2)                 # [par, hh, b, t]
        g = g.transpose(2, 3, 0, 1).reshape(BL, N, H)   # [b, 2t+par, hh]
        outs.append(g)
    return np.concatenate(outs, axis=0).astype(np.float32)


if __name__ == "__main__":
    rng = np.random.default_rng(0)
    inputs = {
        "input_tensor": rng.standard_normal((B, N), dtype=np.float32),
        "hidden": rng.standard_normal((B, N, H), dtype=np.float32),
        "adj": rng.random((N, N), dtype=np.float32),
        "W1": rng.standard_normal((H + 1, 2 * H), dtype=np.float32) * 0.15,
        "b1": np.full((2 * H,), 0.4, np.float32),
        "W2": rng.standard_normal((H + 1, H), dtype=np.float32) * 0.15,
        "b2": np.full((H,), 0.6, np.float32),
    }
    out = kernel(**inputs)
    print(out.shape, out.dtype)


# revision 28
# speedup vs baseline: 1.1974x; 1.1025x over previous
"""GCN-GRU cell fused Trainium2 kernel (8-core data parallel), v2.

Math (per batch b):
    A = d * (adj+I).T * d,  d = rowsum(adj+I)^-0.5
    sig   = sigmoid(A @ [input, hidden] @ W1 + b1)   (N, 128)
    r, u  = node-split halves of sig (nodes <1024 / >=1024)
    c     = tanh(A @ [input, r*hidden] @ W2 + b2)
    out   = u * hidden + (1-u) * c

Structure (per core, 8 batches):
  - BG (big GEMM) feature-major: lhsT = X columns (features), rhs = A
    columns (nodes); PSUM out [feat, m], accumulated over 16 k-tiles.
  - W-GEMMs gate-major weight-stationary: lhsT = W (dup-row layout),
    rhs = AX chunk [feat, 512]; out [gates, m] in PSUM -> activation.
  - sig r-half bounced through DRAM + xbar dma transpose -> node-major
    for the x2 = r*x1 assembly (DVE).  Write+transpose share a queue
    (sync for even b, scalar for odd b) so FIFO order guarantees RAW.
  - W2 gates duplicated (cols doubled) so final gating runs f-major on
    DVE with stride-2 column views; out written f-major [f,b,t]; host
    does the final transpose to [b,n,h] (free).
  - Optional fp8e4 DoubleRow big GEMMs (FP8_BG1/FP8_BG2 env flags).
"""

import os
import numpy as np
import ml_dtypes
from contextlib import ExitStack

import concourse.bacc as bacc
import concourse.mybir as mybir
import concourse.tile as tile
from concourse.bass import ts, ds, broadcast_tensor_aps
from concourse.bass_utils import run_bass_kernel_spmd

P = 128
N = 2048
B = 64
H = 64
NCORES = 8
BL = B // NCORES          # 8 batches per core
KT = N // P               # 16 contraction tiles
CH = N // 512             # 4 output chunks of 512
F32 = mybir.dt.float32
BF16 = mybir.dt.bfloat16
FP8 = mybir.dt.float8e4
SIG = mybir.ActivationFunctionType.Sigmoid
TANH = mybir.ActivationFunctionType.Tanh
DR = mybir.MatmulPerfMode.DoubleRow

S_A = 16.0   # fp8 scale on A side
S_X = 32.0   # fp8 scale on X side

FP8_BG1 = os.environ.get("FP8_BG1", "1") == "1"
FP8_BG2 = os.environ.get("FP8_BG2", "1") == "1"

_CACHE = {}


def _build(fp8_bg1: bool, fp8_bg2: bool):
    nc = bacc.Bacc("TRN2", target_bir_lowering=False)

    any_bf = (not fp8_bg1) or (not fp8_bg2)
    any_f8 = fp8_bg1 or fp8_bg2
    two_dreps = fp8_bg1 != fp8_bg2

    # ---- dram tensors ----
    if any_bf:
        a_d = nc.dram_tensor("a", [N, N], BF16, kind="ExternalInput")
    if any_f8:
        a8_d = nc.dram_tensor("a8", [N, N], FP8, kind="ExternalInput")
    x1_d = nc.dram_tensor("x1", [N, BL * H], BF16, kind="ExternalInput")
    if fp8_bg1:
        x18_d = nc.dram_tensor("x18", [N, BL * H], FP8, kind="ExternalInput")
        xin8_d = nc.dram_tensor("xin8", [P, KT * 16], FP8, kind="ExternalInput")
    else:
        xin_d = nc.dram_tensor("xin", [P, KT * BL], BF16, kind="ExternalInput")
    drep1_d = nc.dram_tensor("drep1", [P, N], F32, kind="ExternalInput")
    if two_dreps:
        drep2_d = nc.dram_tensor("drep2", [P, N], F32, kind="ExternalInput")
    ht_d = nc.dram_tensor("ht", [P, BL, N // 2], BF16, kind="ExternalInput")
    w1h_d = nc.dram_tensor("w1h", [2 * H, 2 * H], BF16, kind="ExternalInput")
    w1i_d = nc.dram_tensor("w1i", [BL + 1, BL, 2 * H], BF16, kind="ExternalInput")
    w2h_d = nc.dram_tensor("w2h", [2 * H, 2 * H], BF16, kind="ExternalInput")
    w2i_d = nc.dram_tensor("w2i", [BL + 1, BL, 2 * H], BF16, kind="ExternalInput")
    out_d = nc.dram_tensor("out", [P, BL, N // 2], BF16, kind="ExternalOutput")
    # scratch for the r-half transpose bounce: one tensor per b so
    # whole-tensor DRAM dep tracking doesn't serialize across batches
    sigr_ds = [
        nc.dram_tensor(f"sigr{b}", [P, N // 2], BF16, kind="Internal")
        for b in range(BL)
    ]

    x2dt = FP8 if fp8_bg2 else BF16

    with tile.TileContext(nc) as tc, ExitStack() as ctx:
        const = ctx.enter_context(tc.tile_pool(name="const", bufs=1))
        x1_sb = const.tile([P, KT, BL * H], BF16)
        if fp8_bg1:
            x18_sb = const.tile([P, KT, BL * H], FP8)
            xin_sb = const.tile([P, KT, 16], FP8)
        else:
            xin_sb = const.tile([P, KT, BL], BF16)
        drep1_sb = const.tile([P, N], F32)
        drep2_sb = const.tile([P, N], F32) if two_dreps else drep1_sb
        ht_sb = const.tile([P, BL, N // 2], BF16)
        w1h_sb = const.tile([2 * H, 2 * H], BF16)
        w1i_sb = const.tile([BL + 1, BL, 2 * H], BF16)
        w2h_sb = const.tile([2 * H, 2 * H], BF16)
        w2i_sb = const.tile([BL + 1, BL, 2 * H], BF16)
        axin_sb = const.tile([BL + 1, N], BF16)     # rows 0..7 = d*(A@din), row 8 = ones
        axf1_sb = const.tile([P, CH, 4, 512], BF16)   # [2b x 64feat, ch, pair, m]
        axf2_sb = const.tile([P, CH, 4, 512], x2dt)
        sigu_sb = const.tile([P, BL, N // 2], BF16)    # [gate, b, m-1024]
        sigrt_sb = const.tile([P, KT, BL, H], BF16)    # node-major r gates
        x2_sb = const.tile([P, KT, BL * H], x2dt)
        c3_sb = const.tile([P, BL, N // 2], BF16)      # de-interleaved c, [f, b, t]

        x1_r = x1_d.ap().rearrange("(kt p) f -> p kt f", p=P)
        if fp8_bg1:
            x18_r = x18_d.ap().rearrange("(kt p) f -> p kt f", p=P)
            for g in range(8):
                nc.scalar.dma_start(x18_sb[:, ts(g, 2), :], x18_r[:, ts(g, 2), :])
            nc.scalar.dma_start(
                xin_sb[:], xin8_d.ap().rearrange("p (kt b) -> p kt b", b=16)
            )
        else:
            nc.scalar.dma_start(
                xin_sb[:], xin_d.ap().rearrange("p (kt b) -> p kt b", b=BL)
            )
        nc.scalar.dma_start(drep1_sb[:], drep1_d.ap())
        if two_dreps:
            nc.scalar.dma_start(drep2_sb[:], drep2_d.ap())
        nc.scalar.dma_start(w1h_sb[:], w1h_d.ap())
        nc.scalar.dma_start(w1i_sb[:], w1i_d.ap())
        nc.vector.memset(axin_sb[:], 1.0)
        for g in range(4):
            nc.scalar.dma_start(x1_sb[:, ts(g, 4), :], x1_r[:, ts(g, 4), :])

        both_f8 = fp8_bg1 and fp8_bg2
        if both_f8:
            a8_sb = const.tile([P, KT, N], FP8)
        apool = ctx.enter_context(tc.tile_pool(name="ap", bufs=3))
        spool = ctx.enter_context(tc.tile_pool(name="sp", bufs=4))
        gpool = ctx.enter_context(tc.tile_pool(name="gp", bufs=2))
        pps = ctx.enter_context(tc.tile_pool(name="ps", bufs=8, space="PSUM"))

        if any_bf:
            a_r = a_d.ap().rearrange("(kt p) m -> p kt m", p=P)
        if any_f8:
            a8_r = a8_d.ap().rearrange("(kt p) m -> p kt m", p=P)

        def load_a_chunk(ch, fp8, fine=False):
            if both_f8:
                at = a8_sb[:, :, ds(ch * 512, 512)]
                src = a8_r[:, :, ds(ch * 512, 512)]
                if fine:
                    for g in range(4):
                        nc.sync.dma_start(at[:, ts(g, 4), :], src[:, ts(g, 4), :])
                else:
                    nc.sync.dma_start(at[:], src)
                return at
            dt = FP8 if fp8 else BF16
            at = apool.tile([P, KT, 512], dt, tag="a8" if fp8 else "abf")
            src = (a8_r if fp8 else a_r)[:, :, ds(ch * 512, 512)]
            if fine:
                for g in range(4):
                    nc.sync.dma_start(at[:, ts(g, 4), :], src[:, ts(g, 4), :])
            else:
                nc.sync.dma_start(at[:], src)
            return at

        def big_gemm(ch, at, xsb, xinsb, axf, dsb, with_in, fp8):
            """Feature-major BG chunk: psum tiles [128, 512] per pair + xin."""
            n_ps = 5 if with_in else 4
            ps = [
                pps.tile([P, 512], F32, tag="ps", name=f"ps{i}") for i in range(n_ps)
            ]
            if fp8:
                for ktp in range(KT // 2):
                    st, sp = ktp == 0, ktp == KT // 2 - 1
                    rhs = at[:, 2 * ktp : 2 * ktp + 2, :]
                    for mf in range(4):
                        nc.tensor.matmul(
                            ps[mf][:],
                            lhsT=xsb[:, 2 * ktp : 2 * ktp + 2, ts(mf, P)],
                            rhs=rhs, start=st, stop=sp, perf_mode=DR,
                        )
                    if with_in:
                        nc.tensor.matmul(
                            ps[4][:16],
                            lhsT=xinsb[:, 2 * ktp : 2 * ktp + 2, :],
                            rhs=rhs, start=st, stop=sp, perf_mode=DR,
                        )
            else:
                for kt in range(KT):
                    st, sp = kt == 0, kt == KT - 1
                    rhs = at[:, kt, :]
                    for mf in range(4):
                        nc.tensor.matmul(
                            ps[mf][:],
                            lhsT=xsb[:, kt, ts(mf, P)],
                            rhs=rhs, start=st, stop=sp,
                        )
                    if with_in:
                        nc.tensor.matmul(
                            ps[4][:BL],
                            lhsT=xinsb[:, kt, :],
                            rhs=rhs, start=st, stop=sp,
                        )
            dcol = dsb[:, ds(ch * 512, 512)]
            for mf in range(4):
                nc.vector.tensor_mul(axf[:, ch, mf, :], ps[mf][:], dcol)
            if with_in:
                nc.vector.tensor_mul(
                    axin_sb[:BL, ds(ch * 512, 512)], ps[4][:BL], dcol[:BL]
                )

        # ---------------- GCN1 ----------------
        a1_tiles = [load_a_chunk(ch, fp8_bg1, fine=(ch == 0)) for ch in range(CH)]

        def w1_chunk(ch):
            """Gate-major W1 for chunk ch; r-half -> dram bounce, u -> sbuf."""
            pws = []
            for b in range(BL):
                pw = pps.tile([P, 512], F32, tag="ps", name=f"pw{b}")
                pws.append(pw)
                jo = 64 * (b % 2)
                nc.tensor.matmul(
                    pw[:],
                    lhsT=w1h_sb[jo : jo + 64, :],
                    rhs=axf1_sb[jo : jo + 64, ch, b // 2, :],
                    start=True, stop=False,
                )
            for b in range(BL):
                q = nc.gpsimd
                pw = pws[b]
                nc.tensor.matmul(
                    pw[:],
                    lhsT=w1i_sb[:, b, :],
                    rhs=axin_sb[:, ds(ch * 512, 512)],
                    start=False, stop=True,
                )
                if ch < 2:
                    sg = spool.tile([P, 512], BF16, tag="sg")
                    nc.scalar.activation(sg[:], pw[:], SIG)
                    q.dma_start(sigr_ds[b].ap()[:, ds(ch * 512, 512)], sg[:])
                    if ch == 1:
                        # coarse xbar transposes: [64,1024] -> [128, 8, 64]
                        for jh in (0, 1):
                            nc.scalar.dma_start_transpose(
                                sigrt_sb[:, ds(8 * jh, 8), b, :],
                                sigr_ds[b].ap()[ds(64 * jh, 64), :],
                            )
                else:
                    nc.scalar.activation(
                        sigu_sb[:, b, ds((ch - 2) * 512, 512)], pw[:], SIG
                    )

        a2_tiles = [None] * CH
        for ch in range(CH):
            big_gemm(ch, a1_tiles[ch], x18_sb if fp8_bg1 else x1_sb,
                     xin_sb, axf1_sb, drep1_sb, True, fp8_bg1)
            w1_chunk(ch)
            if not both_f8:
                if ch == 1:
                    a2_tiles[0] = load_a_chunk(0, fp8_bg2)
                    a2_tiles[1] = load_a_chunk(1, fp8_bg2)
                if ch == 3:
                    a2_tiles[2] = load_a_chunk(2, fp8_bg2)
                    a2_tiles[3] = load_a_chunk(3, fp8_bg2)

        if both_f8:
            a2_tiles = a1_tiles
        # late-needed tensors: load off the critical window
        nc.sync.dma_start(w2h_sb[:], w2h_d.ap())
        nc.sync.dma_start(w2i_sb[:], w2i_d.ap())
        nc.sync.dma_start(ht_sb[:], ht_d.ap())

        # x2 assembly (node-major): x2 = sig_rT * x1, 4 kt per op
        for ktg in range(4):
            s3 = sigrt_sb[:, ts(ktg, 4), :, :].rearrange("p k b h -> p (k b h)")
            x1v = x1_sb[:, ts(ktg, 4), :].rearrange("p k f -> p (k f)")
            x2v = x2_sb[:, ts(ktg, 4), :].rearrange("p k f -> p (k f)")
            nc.vector.tensor_mul(x2v, s3, x1v)

        def w2_chunk(ch):
            pws = []
            for b in range(BL):
                pw = pps.tile([P, 512], F32, tag="ps", name=f"pw2{b}")
                pws.append(pw)
                jo = 64 * (b % 2)
                nc.tensor.matmul(
                    pw[:],
                    lhsT=w2h_sb[jo : jo + 64, :],
                    rhs=axf2_sb[jo : jo + 64, ch, b // 2, :],
                    start=True, stop=False,
                )
            for b in range(BL):
                pw = pws[b]
                nc.tensor.matmul(
                    pw[:],
                    lhsT=w2i_sb[:, b, :],
                    rhs=axin_sb[:, ds(ch * 512, 512)],
                    start=False, stop=True,
                )
                # de-interleave: c3[64*par+hh, b, t] = c[hh, 2t+par]
                for par in (0, 1):
                    pr = ds(64 * par, 64)
                    nc.scalar.activation(
                        c3_sb[pr, b, ds(256 * ch, 256)],
                        pw[pr, par : par + 511 : 2], TANH,
                    )

        def gate_rng(lo, sz):
            """Gating for t in [lo, lo+sz) over all b, f-major; one out DMA."""
            tsl = ds(lo, sz)
            gt = gpool.tile([P, BL, 512], BF16, tag="g", name="gt")
            for b in range(BL):
                tmpt = gpool.tile([P, 512], BF16, tag="tmp", name="tmpt")
                tmp = tmpt[:, :sz]
                cv = c3_sb[:, b, tsl]
                nc.vector.tensor_sub(tmp[:], ht_sb[:, b, tsl], cv)
                nc.vector.tensor_mul(tmp[:], sigu_sb[:, b, tsl], tmp[:])
                nc.vector.tensor_add(gt[:, b, :sz], tmp[:], cv)
            nc.sync.dma_start(out_d.ap()[:, :, tsl], gt[:, :, :sz])

        for ch in range(CH):
            big_gemm(ch, a2_tiles[ch], x2_sb, None, axf2_sb, drep2_sb,
                     False, fp8_bg2)
            w2_chunk(ch)
            if ch == 2:
                gate_rng(0, 512)      # th0: needs c3 ch0+ch1
                gate_rng(512, 256)    # q2: needs c3 ch2
            elif ch == 3:
                gate_rng(768, 256)    # q3: needs c3 ch3

    nc.finalize()
    return nc


def _prep_inputs(input_tensor, hidden, adj, W1, b1, W2, b2, fp8_bg1, fp8_bg2):
    f32 = np.float32
    bf16 = ml_dtypes.bfloat16
    fp8 = ml_dtypes.float8_e4m3fn
    input_tensor = np.ascontiguousarray(input_tensor, f32)
    hidden = np.ascontiguousarray(hidden, f32)
    adj = np.ascontiguousarray(adj, f32)

    pi = np.concatenate([np.arange(0, N, 2), np.arange(1, N, 2)])
    deg = 1.0 + adj.sum(axis=1, dtype=np.float64)
    d = (deg ** -0.5).astype(f32)
    a_full = (adj + np.eye(N, dtype=f32))[pi]

    any_bf = (not fp8_bg1) or (not fp8_bg2)
    any_f8 = fp8_bg1 or fp8_bg2
    two_dreps = fp8_bg1 != fp8_bg2
    shared = {}
    if any_bf:
        shared["a"] = np.ascontiguousarray(a_full).astype(bf16)
    if any_f8:
        shared["a8"] = np.ascontiguousarray(a_full * S_A).astype(fp8)

    sx1 = S_X if any_f8 else 1.0
    sa1 = S_A if fp8_bg1 else 1.0
    sa2 = S_A if fp8_bg2 else 1.0
    shared["drep1"] = np.ascontiguousarray(
        np.broadcast_to(d / (sa1 * sx1), (P, N)), f32
    )
    if two_dreps:
        shared["drep2"] = np.ascontiguousarray(
            np.broadcast_to(d / (sa2 * sx1), (P, N)), f32
        )

    w1h = np.ascontiguousarray(np.concatenate([W1[1:], W1[1:]], 0).astype(bf16))
    w1i = np.zeros((BL + 1, BL, 2 * H), bf16)
    for bb in range(BL):
        w1i[bb, bb, :] = W1[0].astype(bf16)
        w1i[BL, bb, :] = b1.astype(bf16)
    W2h = W2[1:]
    w2d = np.concatenate([W2h, W2h], 1)          # [64, 128] dup cols
    w2h = np.ascontiguousarray(np.concatenate([w2d, w2d], 0).astype(bf16))
    w2i = np.zeros((BL + 1, BL, 2 * H), bf16)
    for bb in range(BL):
        w2i[bb, bb, :] = np.concatenate([W2[0], W2[0]]).astype(bf16)
        w2i[BL, bb, :] = np.concatenate([b2, b2]).astype(bf16)

    dh = d[None, :, None] * hidden          # (B, N, H)
    din = d[None, :] * input_tensor         # (B, N)

    in_maps = []
    for c in range(NCORES):
        bs = slice(BL * c, BL * c + BL)
        x1f = np.ascontiguousarray(
            dh[bs][:, pi, :].transpose(1, 0, 2).reshape(N, BL * H)
        ) * sx1
        xinf = np.ascontiguousarray(
            din[bs][:, pi].T.reshape(KT, P, BL).transpose(1, 0, 2).reshape(P, KT * BL)
        ) * sx1
        # hidden f-major: ht[64*par+hh, b, t] = hidden[b, 2t+par, hh]
        hv = hidden[bs].reshape(BL, N // 2, 2, H)      # [b, t, par, hh]
        ht = np.ascontiguousarray(
            hv.transpose(2, 3, 0, 1).reshape(P, BL, N // 2)
        ).astype(bf16)
        m = {
            "x1": x1f.astype(bf16), "ht": ht,
            "w1h": w1h, "w1i": w1i, "w2h": w2h, "w2i": w2i,
        }
        m.update(shared)
        if fp8_bg1:
            m["x18"] = x1f.astype(fp8)
            xin8 = np.zeros((P, KT, 16), f32)
            xin8[:, :, :BL] = xinf.reshape(P, KT, BL)
            m["xin8"] = np.ascontiguousarray(xin8.reshape(P, KT * 16)).astype(fp8)
        else:
            m["xin"] = xinf.astype(bf16)
        in_maps.append(m)
    return in_maps


LAST_RESULTS = None


def kernel(input_tensor, hidden, adj, W1, b1, W2, b2):
    global LAST_RESULTS
    key = (FP8_BG1, FP8_BG2)
    if key not in _CACHE:
        _CACHE[key] = _build(*key)
    nc = _CACHE[key]
    in_maps = _prep_inputs(input_tensor, hidden, adj, W1, b1, W2, b2, *key)
    res = run_bass_kernel_spmd(nc, in_maps, core_ids=list(range(NCORES)))
    LAST_RESULTS = res
    outs = []
    for r in res.results:
        g = np.asarray(r["out"], np.float32)           # [128, 8, 1024]
        g = g.reshape(2, H, BL, N // 2)                 # [par, hh, b, t]
        g = g.transpose(2, 3, 0, 1).reshape(BL, N, H)   # [b, 2t+par, hh]
        outs.append(g)
    return np.concatenate(outs, axis=0).astype(np.float32)


if __name__ == "__main__":
    rng = np.random.default_rng(0)
    inputs = {
        "input_tensor": rng.standard_normal((B, N), dtype=np.float32),
        "hidden": rng.standard_normal((B, N, H), dtype=np.float32),
        "adj": rng.random((N, N), dtype=np.float32),
        "W1": rng.standard_normal((H + 1, 2 * H), dtype=np.float32) * 0.15,
        "b1": np.full((2 * H,), 0.4, np.float32),
        "W2": rng.standard_normal((H + 1, H), dtype=np.float32) * 0.15,
        "b2": np.full((H,), 0.6, np.float32),
    }
    out = kernel(**inputs)
    print(out.shape, out.dtype)


# revision 30
# speedup vs baseline: 1.1979x; 1.0004x over previous
"""GCN-GRU cell fused Trainium2 kernel (8-core data parallel), v2.

Math (per batch b):
    A = d * (adj+I).T * d,  d = rowsum(adj+I)^-0.5
    sig   = sigmoid(A @ [input, hidden] @ W1 + b1)   (N, 128)
    r, u  = node-split halves of sig (nodes <1024 / >=1024)
    c     = tanh(A @ [input, r*hidden] @ W2 + b2)
    out   = u * hidden + (1-u) * c

Structure (per core, 8 batches):
  - BG (big GEMM) feature-major: lhsT = X columns (features), rhs = A
    columns (nodes); PSUM out [feat, m], accumulated over 16 k-tiles.
  - W-GEMMs gate-major weight-stationary: lhsT = W (dup-row layout),
    rhs = AX chunk [feat, 512]; out [gates, m] in PSUM -> activation.
  - sig r-half bounced through DRAM + xbar dma transpose -> node-major
    for the x2 = r*x1 assembly (DVE).  Write+transpose share a queue
    (sync for even b, scalar for odd b) so FIFO order guarantees RAW.
  - W2 gates duplicated (cols doubled) so final gating runs f-major on
    DVE with stride-2 column views; out written f-major [f,b,t]; host
    does the final transpose to [b,n,h] (free).
  - Optional fp8e4 DoubleRow big GEMMs (FP8_BG1/FP8_BG2 env flags).
"""

import os
import numpy as np
import ml_dtypes
from contextlib import ExitStack

import concourse.bacc as bacc
import concourse.mybir as mybir
import concourse.tile as tile
from concourse.bass import ts, ds, broadcast_tensor_aps
from concourse.bass_utils import run_bass_kernel_spmd

P = 128
N = 2048
B = 64
H = 64
NCORES = 8
BL = B // NCORES          # 8 batches per core
KT = N // P               # 16 contraction tiles
CH = N // 512             # 4 output chunks of 512
F32 = mybir.dt.float32
BF16 = mybir.dt.bfloat16
FP8 = mybir.dt.float8e4
SIG = mybir.ActivationFunctionType.Sigmoid
TANH = mybir.ActivationFunctionType.Tanh
DR = mybir.MatmulPerfMode.DoubleRow

S_A = 16.0   # fp8 scale on A side
S_X = 32.0   # fp8 scale on X side

FP8_BG1 = os.environ.get("FP8_BG1", "1") == "1"
FP8_BG2 = os.environ.get("FP8_BG2", "1") == "1"

_CACHE = {}


def _build(fp8_bg1: bool, fp8_bg2: bool):
    nc = bacc.Bacc("TRN2", target_bir_lowering=False)

    any_bf = (not fp8_bg1) or (not fp8_bg2)
    any_f8 = fp8_bg1 or fp8_bg2
    two_dreps = fp8_bg1 != fp8_bg2

    # ---- dram tensors ----
    if any_bf:
        a_d = nc.dram_tensor("a", [N, N], BF16, kind="ExternalInput")
    if any_f8:
        a8_d = nc.dram_tensor("a8", [N, N], FP8, kind="ExternalInput")
    x1_d = nc.dram_tensor("x1", [N, BL * H], BF16, kind="ExternalInput")
    if fp8_bg1:
        x18_d = nc.dram_tensor("x18", [N, BL * H], FP8, kind="ExternalInput")
        xin8_d = nc.dram_tensor("xin8", [P, KT * 16], FP8, kind="ExternalInput")
    else:
        xin_d = nc.dram_tensor("xin", [P, KT * BL], BF16, kind="ExternalInput")
    drep1_d = nc.dram_tensor("drep1", [P, N], F32, kind="ExternalInput")
    if two_dreps:
        drep2_d = nc.dram_tensor("drep2", [P, N], F32, kind="ExternalInput")
    ht_d = nc.dram_tensor("ht", [P, BL, N // 2], BF16, kind="ExternalInput")
    w1h_d = nc.dram_tensor("w1h", [2 * H, 2 * H], BF16, kind="ExternalInput")
    w1i_d = nc.dram_tensor("w1i", [BL + 1, BL, 2 * H], BF16, kind="ExternalInput")
    w2h_d = nc.dram_tensor("w2h", [2 * H, 2 * H], BF16, kind="ExternalInput")
    w2i_d = nc.dram_tensor("w2i", [BL + 1, BL, 2 * H], BF16, kind="ExternalInput")
    out_d = nc.dram_tensor("out", [P, BL, N // 2], BF16, kind="ExternalOutput")
    # scratch for the r-half transpose bounce: one tensor per b so
    # whole-tensor DRAM dep tracking doesn't serialize across batches
    sigr_ds = [
        nc.dram_tensor(f"sigr{b}", [P, N // 2], BF16, kind="Internal")
        for b in range(BL)
    ]

    x2dt = FP8 if fp8_bg2 else BF16

    with tile.TileContext(nc) as tc, ExitStack() as ctx:
        const = ctx.enter_context(tc.tile_pool(name="const", bufs=1))
        x1_sb = const.tile([P, KT, BL * H], BF16)
        if fp8_bg1:
            x18_sb = const.tile([P, KT, BL * H], FP8)
            xin_sb = const.tile([P, KT, 16], FP8)
        else:
            xin_sb = const.tile([P, KT, BL], BF16)
        drep1_sb = const.tile([P, N], F32)
        drep2_sb = const.tile([P, N], F32) if two_dreps else drep1_sb
        ht_sb = const.tile([P, BL, N // 2], BF16)
        w1h_sb = const.tile([2 * H, 2 * H], BF16)
        w1i_sb = const.tile([BL + 1, BL, 2 * H], BF16)
        w2h_sb = const.tile([2 * H, 2 * H], BF16)
        w2i_sb = const.tile([BL + 1, BL, 2 * H], BF16)
        axin_sb = const.tile([BL + 1, N], BF16)     # rows 0..7 = d*(A@din), row 8 = ones
        axf1_sb = const.tile([P, CH, 4, 512], BF16)   # [2b x 64feat, ch, pair, m]
        axf2_sb = const.tile([P, CH, 4, 512], x2dt)
        sigu_sb = const.tile([P, BL, N // 2], BF16)    # [gate, b, m-1024]
        sigrt_sb = const.tile([P, KT, BL, H], BF16)    # node-major r gates
        x2_sb = const.tile([P, KT, BL * H], x2dt)
        c3_sb = const.tile([P, BL, N // 2], BF16)      # de-interleaved c, [f, b, t]

        x1_r = x1_d.ap().rearrange("(kt p) f -> p kt f", p=P)
        if fp8_bg1:
            x18_r = x18_d.ap().rearrange("(kt p) f -> p kt f", p=P)
            for g in range(8):
                nc.scalar.dma_start(x18_sb[:, ts(g, 2), :], x18_r[:, ts(g, 2), :])
            nc.scalar.dma_start(
                xin_sb[:], xin8_d.ap().rearrange("p (kt b) -> p kt b", b=16)
            )
        else:
            nc.scalar.dma_start(
                xin_sb[:], xin_d.ap().rearrange("p (kt b) -> p kt b", b=BL)
            )
        nc.scalar.dma_start(drep1_sb[:], drep1_d.ap())
        if two_dreps:
            nc.scalar.dma_start(drep2_sb[:], drep2_d.ap())
        nc.scalar.dma_start(w1h_sb[:], w1h_d.ap())
        nc.scalar.dma_start(w1i_sb[:], w1i_d.ap())
        nc.vector.memset(axin_sb[:], 1.0)
        for g in range(4):
            nc.scalar.dma_start(x1_sb[:, ts(g, 4), :], x1_r[:, ts(g, 4), :])

        both_f8 = fp8_bg1 and fp8_bg2
        if both_f8:
            a8_sb = const.tile([P, KT, N], FP8)
        apool = ctx.enter_context(tc.tile_pool(name="ap", bufs=3))
        spool = ctx.enter_context(tc.tile_pool(name="sp", bufs=4))
        gpool = ctx.enter_context(tc.tile_pool(name="gp", bufs=2))
        pps = ctx.enter_context(tc.tile_pool(name="ps", bufs=8, space="PSUM"))

        if any_bf:
            a_r = a_d.ap().rearrange("(kt p) m -> p kt m", p=P)
        if any_f8:
            a8_r = a8_d.ap().rearrange("(kt p) m -> p kt m", p=P)

        def load_a_chunk(ch, fp8, fine=False):
            if both_f8:
                at = a8_sb[:, :, ds(ch * 512, 512)]
                src = a8_r[:, :, ds(ch * 512, 512)]
                if fine:
                    for g in range(4):
                        nc.sync.dma_start(at[:, ts(g, 4), :], src[:, ts(g, 4), :])
                else:
                    nc.sync.dma_start(at[:], src)
                return at
            dt = FP8 if fp8 else BF16
            at = apool.tile([P, KT, 512], dt, tag="a8" if fp8 else "abf")
            src = (a8_r if fp8 else a_r)[:, :, ds(ch * 512, 512)]
            if fine:
                for g in range(4):
                    nc.sync.dma_start(at[:, ts(g, 4), :], src[:, ts(g, 4), :])
            else:
                nc.sync.dma_start(at[:], src)
            return at

        def big_gemm(ch, at, xsb, xinsb, axf, dsb, with_in, fp8):
            """Feature-major BG chunk: psum tiles [128, 512] per pair + xin."""
            n_ps = 5 if with_in else 4
            ps = [
                pps.tile([P, 512], F32, tag="ps", name=f"ps{i}") for i in range(n_ps)
            ]
            if fp8:
                for ktp in range(KT // 2):
                    st, sp = ktp == 0, ktp == KT // 2 - 1
                    rhs = at[:, 2 * ktp : 2 * ktp + 2, :]
                    for mf in range(4):
                        nc.tensor.matmul(
                            ps[mf][:],
                            lhsT=xsb[:, 2 * ktp : 2 * ktp + 2, ts(mf, P)],
                            rhs=rhs, start=st, stop=sp, perf_mode=DR,
                        )
                    if with_in:
                        nc.tensor.matmul(
                            ps[4][:16],
                            lhsT=xinsb[:, 2 * ktp : 2 * ktp + 2, :],
                            rhs=rhs, start=st, stop=sp, perf_mode=DR,
                        )
            else:
                for kt in range(KT):
                    st, sp = kt == 0, kt == KT - 1
                    rhs = at[:, kt, :]
                    for mf in range(4):
                        nc.tensor.matmul(
                            ps[mf][:],
                            lhsT=xsb[:, kt, ts(mf, P)],
                            rhs=rhs, start=st, stop=sp,
                        )
                    if with_in:
                        nc.tensor.matmul(
                            ps[4][:BL],
                            lhsT=xinsb[:, kt, :],
                            rhs=rhs, start=st, stop=sp,
                        )
            dcol = dsb[:, ds(ch * 512, 512)]
            for mf in range(4):
                nc.vector.tensor_mul(axf[:, ch, mf, :], ps[mf][:], dcol)
            if with_in:
                nc.vector.tensor_mul(
                    axin_sb[:BL, ds(ch * 512, 512)], ps[4][:BL], dcol[:BL]
                )

        # ---------------- GCN1 ----------------
        a1_tiles = [load_a_chunk(ch, fp8_bg1, fine=(ch == 0)) for ch in range(CH)]

        def w1_chunk(ch):
            """Gate-major W1 for chunk ch; r-half -> dram bounce, u -> sbuf."""
            pws = []
            for b in range(BL):
                pw = pps.tile([P, 512], F32, tag="ps", name=f"pw{b}")
                pws.append(pw)
                jo = 64 * (b % 2)
                nc.tensor.matmul(
                    pw[:],
                    lhsT=w1h_sb[jo : jo + 64, :],
                    rhs=axf1_sb[jo : jo + 64, ch, b // 2, :],
                    start=True, stop=False,
                )
            for b in range(BL):
                q = nc.gpsimd
                pw = pws[b]
                nc.tensor.matmul(
                    pw[:],
                    lhsT=w1i_sb[:, b, :],
                    rhs=axin_sb[:, ds(ch * 512, 512)],
                    start=False, stop=True,
                )
                if ch < 2:
                    sg = spool.tile([P, 512], BF16, tag="sg")
                    nc.scalar.activation(sg[:], pw[:], SIG)
                    q.dma_start(sigr_ds[b].ap()[:, ds(ch * 512, 512)], sg[:])
                    if ch == 1:
                        # coarse xbar transposes: [64,1024] -> [128, 8, 64]
                        for jh in (0, 1):
                            nc.scalar.dma_start_transpose(
                                sigrt_sb[:, ds(8 * jh, 8), b, :],
                                sigr_ds[b].ap()[ds(64 * jh, 64), :],
                            )
                else:
                    nc.scalar.activation(
                        sigu_sb[:, b, ds((ch - 2) * 512, 512)], pw[:], SIG
                    )

        a2_tiles = [None] * CH
        for ch in range(CH):
            big_gemm(ch, a1_tiles[ch], x18_sb if fp8_bg1 else x1_sb,
                     xin_sb, axf1_sb, drep1_sb, True, fp8_bg1)
            w1_chunk(ch)
            if not both_f8:
                if ch == 1:
                    a2_tiles[0] = load_a_chunk(0, fp8_bg2)
                    a2_tiles[1] = load_a_chunk(1, fp8_bg2)
                if ch == 3:
                    a2_tiles[2] = load_a_chunk(2, fp8_bg2)
                    a2_tiles[3] = load_a_chunk(3, fp8_bg2)

        if both_f8:
            a2_tiles = a1_tiles
        # late-needed tensors: load off the critical window
        nc.sync.dma_start(w2h_sb[:], w2h_d.ap())
        nc.sync.dma_start(w2i_sb[:], w2i_d.ap())
        nc.sync.dma_start(ht_sb[:], ht_d.ap())

        # x2 assembly (node-major): x2 = sig_rT * x1, 4 kt per op
        for ktg in range(4):
            s3 = sigrt_sb[:, ts(ktg, 4), :, :].rearrange("p k b h -> p (k b h)")
            x1v = x1_sb[:, ts(ktg, 4), :].rearrange("p k f -> p (k f)")
            x2v = x2_sb[:, ts(ktg, 4), :].rearrange("p k f -> p (k f)")
            nc.vector.tensor_mul(x2v, s3, x1v)

        def w2_chunk(ch):
            pws = []
            nb = 4 if ch == CH - 1 else BL
            for b0 in range(0, BL, nb):
                for b in range(b0, b0 + nb):
                    pw = pps.tile([P, 512], F32, tag="ps", name=f"pw2{b}")
                    pws.append(pw)
                    jo = 64 * (b % 2)
                    nc.tensor.matmul(
                        pw[:],
                        lhsT=w2h_sb[jo : jo + 64, :],
                        rhs=axf2_sb[jo : jo + 64, ch, b // 2, :],
                        start=True, stop=False,
                    )
                for b in range(b0, b0 + nb):
                    pw = pws[b]
                    nc.tensor.matmul(
                        pw[:],
                        lhsT=w2i_sb[:, b, :],
                        rhs=axin_sb[:, ds(ch * 512, 512)],
                        start=False, stop=True,
                    )
                    # de-interleave: c3[64*par+hh, b, t] = c[hh, 2t+par]
                    for par in (0, 1):
                        pr = ds(64 * par, 64)
                        nc.scalar.activation(
                            c3_sb[pr, b, ds(256 * ch, 256)],
                            pw[pr, par : par + 511 : 2], TANH,
                        )

        def gate_rng(lo, sz):
            """Gating for t in [lo, lo+sz) over all b, f-major; one out DMA."""
            tsl = ds(lo, sz)
            gt = gpool.tile([P, BL, 512], BF16, tag="g", name="gt")
            for b in range(BL):
                tmpt = gpool.tile([P, 512], BF16, tag="tmp", name="tmpt")
                tmp = tmpt[:, :sz]
                cv = c3_sb[:, b, tsl]
                nc.vector.tensor_sub(tmp[:], ht_sb[:, b, tsl], cv)
                nc.vector.tensor_mul(tmp[:], sigu_sb[:, b, tsl], tmp[:])
                nc.vector.tensor_add(gt[:, b, :sz], tmp[:], cv)
            nc.sync.dma_start(out_d.ap()[:, :, tsl], gt[:, :, :sz])

        for ch in range(CH):
            big_gemm(ch, a2_tiles[ch], x2_sb, None, axf2_sb, drep2_sb,
                     False, fp8_bg2)
            w2_chunk(ch)
            if ch == 2:
                gate_rng(0, 512)      # th0: needs c3 ch0+ch1
                gate_rng(512, 256)    # q2: needs c3 ch2
            elif ch == 3:
                gate_rng(768, 256)    # q3: needs c3 ch3

    nc.finalize()
    return nc


def _prep_inputs(input_tensor, hidden, adj, W1, b1, W2, b2, fp8_bg1, fp8_bg2):
    f32 = np.float32
    bf16 = ml_dtypes.bfloat16
    fp8 = ml_dtypes.float8_e4m3fn
    input_tensor = np.ascontiguousarray(input_tensor, f32)
    hidden = np.ascontiguousarray(hidden, f32)
    adj = np.ascontiguousarray(adj, f32)

    pi = np.concatenate([np.arange(0, N, 2), np.arange(1, N, 2)])
    deg = 1.0 + adj.sum(axis=1, dtype=np.float64)
    d = (deg ** -0.5).astype(f32)
    a_full = (adj + np.eye(N, dtype=f32))[pi]

    any_bf = (not fp8_bg1) or (not fp8_bg2)
    any_f8 = fp8_bg1 or fp8_bg2
    two_dreps = fp8_bg1 != fp8_bg2
    shared = {}
    if any_bf:
        shared["a"] = np.ascontiguousarray(a_full).astype(bf16)
    if any_f8:
        shared["a8"] = np.ascontiguousarray(a_full * S_A).astype(fp8)

    sx1 = S_X if any_f8 else 1.0
    sa1 = S_A if fp8_bg1 else 1.0
    sa2 = S_A if fp8_bg2 else 1.0
    shared["drep1"] = np.ascontiguousarray(
        np.broadcast_to(d / (sa1 * sx1), (P, N)), f32
    )
    if two_dreps:
        shared["drep2"] = np.ascontiguousarray(
            np.broadcast_to(d / (sa2 * sx1), (P, N)), f32
        )

    w1h = np.ascontiguousarray(np.concatenate([W1[1:], W1[1:]], 0).astype(bf16))
    w1i = np.zeros((BL + 1, BL, 2 * H), bf16)
    for bb in range(BL):
        w1i[bb, bb, :] = W1[0].astype(bf16)
        w1i[BL, bb, :] = b1.astype(bf16)
    W2h = W2[1:]
    w2d = np.concatenate([W2h, W2h], 1)          # [64, 128] dup cols
    w2h = np.ascontiguousarray(np.concatenate([w2d, w2d], 0).astype(bf16))
    w2i = np.zeros((BL + 1, BL, 2 * H), bf16)
    for bb in range(BL):
        w2i[bb, bb, :] = np.concatenate([W2[0], W2[0]]).astype(bf16)
        w2i[BL, bb, :] = np.concatenate([b2, b2]).astype(bf16)

    dh = d[None, :, None] * hidden          # (B, N, H)
    din = d[None, :] * input_tensor         # (B, N)

    in_maps = []
    for c in range(NCORES):
        bs = slice(BL * c, BL * c + BL)
        x1f = np.ascontiguousarray(
            dh[bs][:, pi, :].transpose(1, 0, 2).reshape(N, BL * H)
        ) * sx1
        xinf = np.ascontiguousarray(
            din[bs][:, pi].T.reshape(KT, P, BL).transpose(1, 0, 2).reshape(P, KT * BL)
        ) * sx1
        # hidden f-major: ht[64*par+hh, b, t] = hidden[b, 2t+par, hh]
        hv = hidden[bs].reshape(BL, N // 2, 2, H)      # [b, t, par, hh]
        ht = np.ascontiguousarray(
            hv.transpose(2, 3, 0, 1).reshape(P, BL, N // 2)
        ).astype(bf16)
        m = {
            "x1": x1f.astype(bf16), "ht": ht,
            "w1h": w1h, "w1i": w1i, "w2h": w2h, "w2i": w2i,
        }
        m.update(shared)
        if fp8_bg1:
            m["x18"] = x1f.astype(fp8)
            xin8 = np.zeros((P, KT, 16), f32)
            xin8[:, :, :BL] = xinf.reshape(P, KT, BL)
            m["xin8"] = np.ascontiguousarray(xin8.reshape(P, KT * 16)).astype(fp8)
        else:
            m["xin"] = xinf.astype(bf16)
        in_maps.append(m)
    return in_maps


LAST_RESULTS = None


def kernel(input_tensor, hidden, adj, W1, b1, W2, b2):
    global LAST_RESULTS
    key = (FP8_BG1, FP8_BG2)
    if key not in _CACHE:
        _CACHE[key] = _build(*key)
    nc = _CACHE[key]
    in_maps = _prep_inputs(input_tensor, hidden, adj, W1, b1, W2, b2, *key)
    res = run_bass_kernel_spmd(nc, in_maps, core_ids=list(range(NCORES)))
    LAST_RESULTS = res
    outs = []
    for r in res.results:
        g = np.asarray(r["out"], np.float32)           # [128, 8, 1024]
        g = g.reshape(2, H, BL, N // 2)                 # [par, hh, b, t]
        g = g.transpose(2, 3, 0, 1).reshape(BL, N, H)   # [b, 2t+par, hh]
        outs.append(g)
    return np.concatenate(outs, axis=0).astype(np.float32)


if __name__ == "__main__":
    rng = np.random.default_rng(0)
    inputs = {
        "input_tensor": rng.standard_normal((B, N), dtype=np.float32),
        "hidden": rng.standard_normal((B, N, H), dtype=np.float32),
        "adj": rng.random((N, N), dtype=np.float32),
        "W1": rng.standard_normal((H + 1, 2 * H), dtype=np.float32) * 0.15,
        "b1": np.full((2 * H,), 0.4, np.float32),
        "W2": rng.standard_normal((H + 1, H), dtype=np.float32) * 0.15,
        "b2": np.full((H,), 0.6, np.float32),
    }
    out = kernel(**inputs)
    print(out.shape, out.dtype)
